# revision 1
# baseline (speedup 1.0000x reference)
# Trainium2 Bass kernel for nn_Graph_AutoEncoder (BiLSTM encoder + GRU decoder).
#
# Sharding: decoder rows i in [256c, 256c+256) per core c. Each core encodes the
# 512 batch rows j = 2i, 2i+1 its decoder slice needs (LSTM1 both dirs at B=512,
# LSTM2 one direction). Cores 4-7 need the *backward* LSTM2 direction; they get
# time-reversed edge sequences and f/b-swapped LSTM1 weights via their input map,
# so the compiled program is identical on all 8 cores (SPMD, no collectives).
#
# Layout: feature-on-partition. Gates are computed as W @ h matmuls into PSUM
# (lhsT = W^T with K on partitions), biases folded either into K=2 x-projection
# matmuls (rhs = [x_t; ones]) or into ScalarE activation bias operands.
import numpy as np

import concourse.bass as bass
import concourse.mybir as mybir
import concourse.tile as tile
from concourse import bacc
from concourse.bass_utils import run_bass_kernel_spmd

F32 = mybir.dt.float32
I32 = mybir.dt.int32
SIG = mybir.ActivationFunctionType.Sigmoid
TANH = mybir.ActivationFunctionType.Tanh
MUL = mybir.AluOpType.mult
ADD = mybir.AluOpType.add
SUB = mybir.AluOpType.subtract

T = 200
NC = 8
BE = 512  # encoder batch per core
BD = 256  # decoder batch per core

_CACHE = {}


def _build_program():
    nc = bacc.Bacc("TRN2", target_bir_lowering=False, debug=False, num_devices=NC)

    def din(name, shape, d=F32):
        return nc.dram_tensor(name, shape, d, kind="ExternalInput").ap()

    edge_src = din("edge_src", [T, 2, BE])
    l1_whhT = din("l1_whhT", [2, 128, 512])
    l1_xaug = din("l1_xaug", [2, 2, 512])
    l2_wihT = din("l2_wihT", [2, 128, 512])
    l2_whhT_i = din("l2_whhT", [128, 512])
    l2_bias_i = din("l2_bias", [128, 4])
    fc1_wT = din("fc1_wT", [2, 128, 256])
    fc1_bias_i = din("fc1_bias", [128, 2])
    fc2_wT = din("fc2_wT", [2, 128, 64])
    fc2_bias_i = din("fc2_bias", [64, 1])
    g1_whhT_i = din("g1_whhT", [128, 384])
    g1_xaug_i = din("g1_xaug", [2, 384])
    g1_bhhn_i = din("g1_bhhn", [1, 128])
    g2_wxT_i = din("g2_wxT", [128, 150])
    g2_whhT_i = din("g2_whhT", [50, 150])
    g2_brz_i = din("g2_brz", [1, 100])
    g2_bn_i = din("g2_bn", [50, 1])
    g2_bhhn_i = din("g2_bhhn", [1, 50])
    dec_wTr_i = din("dec_wTr", [50, 128])
    dec_br_i = din("dec_br", [128, 1])
    ones_i = din("ones_in", [1, BE])
    ident_i = din("ident", [128, 128])
    emb_i = din("emb", [50000, 64])
    node_idx_i = din("node_idx", [BD, 2], I32)
    dec_init_i = din("dec_init", [2, BD])

    out_staged = nc.dram_tensor("out_staged", [25, 8 * BD], F32, kind="ExternalOutput").ap()
    h1_buf = nc.dram_tensor("h1_buf", [2, T, 128, BE], F32).ap()

    with tile.TileContext(nc) as tc:
        with (
            tc.tile_pool(name="wpool", bufs=1) as wp,
            tc.tile_pool(name="spool", bufs=1) as sp,
        ):
            # ---- persistent weights ----
            l1w = wp.tile([128, 1024], F32)
            nc.sync.dma_start(out=l1w[:, 0:512], in_=l1_whhT[0])
            nc.sync.dma_start(out=l1w[:, 512:1024], in_=l1_whhT[1])
            l1x = wp.tile([2, 1024], F32)
            nc.sync.dma_start(out=l1x[:, 0:512], in_=l1_xaug[0])
            nc.sync.dma_start(out=l1x[:, 512:1024], in_=l1_xaug[1])
            l2wih = wp.tile([128, 1024], F32)
            nc.sync.dma_start(out=l2wih[:, 0:512], in_=l2_wihT[0])
            nc.sync.dma_start(out=l2wih[:, 512:1024], in_=l2_wihT[1])
            l2whh = wp.tile([128, 512], F32)
            nc.sync.dma_start(out=l2whh[:], in_=l2_whhT_i[:])
            l2b = wp.tile([128, 4], F32)
            nc.sync.dma_start(out=l2b[:], in_=l2_bias_i[:])
            fc1w = wp.tile([128, 512], F32)
            nc.sync.dma_start(out=fc1w[:, 0:256], in_=fc1_wT[0])
            nc.sync.dma_start(out=fc1w[:, 256:512], in_=fc1_wT[1])
            fc1b = wp.tile([128, 2], F32)
            nc.sync.dma_start(out=fc1b[:], in_=fc1_bias_i[:])
            fc2w = wp.tile([128, 128], F32)
            nc.sync.dma_start(out=fc2w[:, 0:64], in_=fc2_wT[0])
            nc.sync.dma_start(out=fc2w[:, 64:128], in_=fc2_wT[1])
            fc2b = wp.tile([64, 1], F32)
            nc.sync.dma_start(out=fc2b[:], in_=fc2_bias_i[:])
            g1whh = wp.tile([128, 384], F32)
            nc.sync.dma_start(out=g1whh[:], in_=g1_whhT_i[:])
            g1x = wp.tile([2, 384], F32)
            nc.sync.dma_start(out=g1x[:], in_=g1_xaug_i[:])
            g1bhhn = wp.tile([1, 128], F32)
            nc.sync.dma_start(out=g1bhhn[:], in_=g1_bhhn_i[:])
            g2wx = wp.tile([128, 150], F32)
            nc.sync.dma_start(out=g2wx[:], in_=g2_wxT_i[:])
            g2whh = wp.tile([50, 150], F32)
            nc.sync.dma_start(out=g2whh[:], in_=g2_whhT_i[:])
            g2brz = wp.tile([1, 100], F32)
            nc.sync.dma_start(out=g2brz[:], in_=g2_brz_i[:])
            g2bn = wp.tile([50, 1], F32)
            nc.sync.dma_start(out=g2bn[:], in_=g2_bn_i[:])
            g2bhhn = wp.tile([1, 50], F32)
            nc.sync.dma_start(out=g2bhhn[:], in_=g2_bhhn_i[:])
            decw = wp.tile([50, 128], F32)
            nc.sync.dma_start(out=decw[:], in_=dec_wTr_i[:])
            decb = wp.tile([128, 1], F32)
            nc.sync.dma_start(out=decb[:], in_=dec_br_i[:])
            ones = wp.tile([1, BE], F32)
            nc.sync.dma_start(out=ones[:], in_=ones_i[:])

            # ---- persistent state ----
            hn1 = sp.tile([128, BE], F32)
            hn2cap = sp.tile([128, BE], F32)
            hinit = sp.tile([128, BD], F32)
            h2g = sp.tile([50, BD], F32)
            res = sp.tile([2, BD], F32)

            # ================= LSTM1 (both dirs, B=512) =================
            with (
                tc.tile_pool(name="l1ring", bufs=8) as rp,
                tc.tile_pool(name="l1hring", bufs=4) as hp,
                tc.tile_pool(name="l1work", bufs=3) as kp,
                tc.tile_pool(name="l1state", bufs=1) as lsp,
                tc.tile_pool(name="l1psum", bufs=1, space="PSUM") as pp,
            ):
                c1 = lsp.tile([128, 1024], F32)
                nc.vector.memset(c1[:], 0.0)
                h_prev = hp.tile([128, 1024], F32, tag="h1o")
                nc.vector.memset(h_prev[:], 0.0)
                for s in range(T):
                    erf = rp.tile([2, BE], F32, tag="erf")
                    nc.sync.dma_start(out=erf[:], in_=edge_src[s])
                    erb = rp.tile([2, BE], F32, tag="erb")
                    nc.sync.dma_start(out=erb[:], in_=edge_src[T - 1 - s])
                    sigp = pp.tile([128, 3072], F32, tag="sigp", space="PSUM")
                    gp = pp.tile([128, 1024], F32, tag="gp", space="PSUM")
                    for d in (0, 1):
                        hs_d = h_prev[:, 512 * d : 512 * d + 512]
                        er_d = erf if d == 0 else erb
                        for gi, g in ((0, 0), (1, 1), (2, 3), (3, 2)):
                            if g == 2:  # tanh gate
                                dst = gp[:, 512 * d : 512 * d + 512]
                            else:
                                col = (0, 1, None, 2)[g]
                                dst = sigp[:, 1024 * col + 512 * d : 1024 * col + 512 * d + 512]
                            nc.tensor.matmul(
                                dst, lhsT=l1w[:, 512 * d + 128 * g : 512 * d + 128 * g + 128],
                                rhs=hs_d, start=True, stop=False)
                            nc.tensor.matmul(
                                dst, lhsT=l1x[:, 512 * d + 128 * g : 512 * d + 128 * g + 128],
                                rhs=er_d[:], start=False, stop=True)
                    sig_sb = kp.tile([128, 3072], F32, tag="sig_sb")
                    nc.scalar.activation(sig_sb[:], sigp[:], SIG)
                    g_sb = kp.tile([128, 1024], F32, tag="g_sb")
                    nc.scalar.activation(g_sb[:], gp[:], TANH)
                    t1 = kp.tile([128, 1024], F32, tag="t1")
                    nc.vector.tensor_tensor(out=t1[:], in0=sig_sb[:, 0:1024], in1=g_sb[:], op=MUL)
                    nc.vector.tensor_tensor(out=c1[:], in0=sig_sb[:, 1024:2048], in1=c1[:], op=MUL)
                    nc.vector.tensor_tensor(out=c1[:], in0=c1[:], in1=t1[:], op=ADD)
                    tc_sb = kp.tile([128, 1024], F32, tag="tc_sb")
                    nc.scalar.activation(tc_sb[:], c1[:], TANH)
                    h_cur = hp.tile([128, 1024], F32, tag="h1o")
                    nc.gpsimd.tensor_tensor(out=h_cur[:], in0=sig_sb[:, 2048:3072], in1=tc_sb[:], op=MUL)
                    nc.sync.dma_start(out=h1_buf[0, s], in_=h_cur[:, 0:512])
                    nc.sync.dma_start(out=h1_buf[1, T - 1 - s], in_=h_cur[:, 512:1024])
                    h_prev = h_cur
                nc.vector.tensor_copy(out=hn1[:], in_=h_prev[:, 0:512])

            # ================= LSTM2 (one dir, B=512) =================
            with (
                tc.tile_pool(name="l2ring", bufs=8) as rp2,
                tc.tile_pool(name="l2work", bufs=3) as kp2,
                tc.tile_pool(name="l2state", bufs=1) as lsp2,
                tc.tile_pool(name="l2psum", bufs=2, space="PSUM") as pp2,
            ):
                c2 = lsp2.tile([128, 512], F32)
                nc.vector.memset(c2[:], 0.0)
                h2p = lsp2.tile([128, 512], F32)
                nc.vector.memset(h2p[:], 0.0)
                h2n = lsp2.tile([128, 512], F32)
                for s in range(T):
                    xf = rp2.tile([128, 512], F32, tag="xf")
                    nc.sync.dma_start(out=xf[:], in_=h1_buf[0, s])
                    xb = rp2.tile([128, 512], F32, tag="xb")
                    nc.sync.dma_start(out=xb[:], in_=h1_buf[1, s])
                    sp2t = pp2.tile([128, 1536], F32, tag="sp2", space="PSUM")
                    gp2 = pp2.tile([128, 512], F32, tag="gp2", space="PSUM")
                    for g, dst_info in ((0, (sp2t, 0)), (1, (sp2t, 512)), (3, (sp2t, 1024)), (2, (gp2, 0))):
                        dtile, off = dst_info
                        dst = dtile[:, off : off + 512]
                        ws = slice(128 * g, 128 * g + 128)
                        nc.tensor.matmul(dst, lhsT=l2wih[:, 128 * g : 128 * g + 128], rhs=xf[:], start=True, stop=False)
                        nc.tensor.matmul(dst, lhsT=l2wih[:, 512 + 128 * g : 512 + 128 * g + 128], rhs=xb[:], start=False, stop=False)
                        nc.tensor.matmul(dst, lhsT=l2whh[:, 128 * g : 128 * g + 128], rhs=h2p[:], start=False, stop=True)
                    sb2 = kp2.tile([128, 1536], F32, tag="sb2")
                    nc.scalar.activation(sb2[:, 0:512], sp2t[:, 0:512], SIG, bias=l2b[:, 0:1])
                    nc.scalar.activation(sb2[:, 512:1024], sp2t[:, 512:1024], SIG, bias=l2b[:, 1:2])
                    nc.scalar.activation(sb2[:, 1024:1536], sp2t[:, 1024:1536], SIG, bias=l2b[:, 3:4])
                    g2sb = kp2.tile([128, 512], F32, tag="g2sb")
                    nc.scalar.activation(g2sb[:], gp2[:], TANH, bias=l2b[:, 2:3])
                    t2 = kp2.tile([128, 512], F32, tag="t2")
                    nc.vector.tensor_tensor(out=t2[:], in0=sb2[:, 0:512], in1=g2sb[:], op=MUL)
                    nc.vector.tensor_tensor(out=c2[:], in0=sb2[:, 512:1024], in1=c2[:], op=MUL)
                    nc.vector.tensor_tensor(out=c2[:], in0=c2[:], in1=t2[:], op=ADD)
                    tc2 = kp2.tile([128, 512], F32, tag="tc2")
                    nc.scalar.activation(tc2[:], c2[:], TANH)
                    dst_h = hn2cap if s == T - 1 else (h2n if s % 2 == 0 else h2p)
                    nc.gpsimd.tensor_tensor(out=dst_h[:], in0=sb2[:, 1024:1536], in1=tc2[:], op=MUL)
                    h2p, h2n = dst_h, (h2p if s % 2 == 0 else h2n)

            # ================= encoder tail =================
            with (
                tc.tile_pool(name="etwork", bufs=1) as ep,
                tc.tile_pool(name="etpsum", bufs=1, space="PSUM") as epp,
            ):
                hnsum = ep.tile([128, BE], F32)
                nc.vector.tensor_tensor(out=hnsum[:], in0=hn1[:], in1=hn2cap[:], op=ADD)
                X = ep.tile([128, 512], F32)
                hv = hnsum[:].rearrange("p (k two) -> p two k", two=2)
                nc.vector.tensor_copy(out=X[:, 0:256], in_=hv[:, 0, :])
                nc.vector.tensor_copy(out=X[:, 256:512], in_=hv[:, 1, :])
                fc1p = epp.tile([128, 512], F32, tag="fc1p", space="PSUM")
                for m in (0, 1):
                    dst = fc1p[:, 256 * m : 256 * m + 256]
                    nc.tensor.matmul(dst, lhsT=fc1w[:, 128 * m : 128 * m + 128], rhs=X[:, 0:256], start=True, stop=False)
                    nc.tensor.matmul(dst, lhsT=fc1w[:, 256 + 128 * m : 256 + 128 * m + 128], rhs=X[:, 256:512], start=False, stop=True)
                Y = ep.tile([128, 512], F32)
                nc.scalar.activation(Y[:, 0:256], fc1p[:, 0:256], SIG, bias=fc1b[:, 0:1])
                nc.scalar.activation(Y[:, 256:512], fc1p[:, 256:512], SIG, bias=fc1b[:, 1:2])
                fc2p = epp.tile([64, 256], F32, tag="fc2p", space="PSUM")
                nc.tensor.matmul(fc2p[:], lhsT=fc2w[:, 0:64], rhs=Y[:, 0:256], start=True, stop=False)
                nc.tensor.matmul(fc2p[:], lhsT=fc2w[:, 64:128], rhs=Y[:, 256:512], start=False, stop=True)
                nc.scalar.activation(hinit[0:64, :], fc2p[:], SIG, bias=fc2b[:])

                # node embedding gather + mean + transpose
                idt = ep.tile([128, 128], F32)
                nc.sync.dma_start(out=idt[:], in_=ident_i[:])
                gsum = []
                for half in (0, 1):
                    rows = slice(128 * half, 128 * half + 128)
                    ih = ep.tile([128, 1], I32, name=f"ih{half}")
                    nc.sync.dma_start(out=ih[:], in_=node_idx_i[rows, 0:1])
                    it = ep.tile([128, 1], I32, name=f"it{half}")
                    nc.sync.dma_start(out=it[:], in_=node_idx_i[rows, 1:2])
                    gh = ep.tile([128, 64], F32, name=f"gh{half}")
                    nc.gpsimd.indirect_dma_start(
                        out=gh[:], out_offset=None, in_=emb_i[:],
                        in_offset=bass.IndirectOffsetOnAxis(ap=ih[:, 0:1], axis=0))
                    gt = ep.tile([128, 64], F32, name=f"gt{half}")
                    nc.gpsimd.indirect_dma_start(
                        out=gt[:], out_offset=None, in_=emb_i[:],
                        in_offset=bass.IndirectOffsetOnAxis(ap=it[:, 0:1], axis=0))
                    sm = ep.tile([128, 64], F32, name=f"sm{half}")
                    nc.vector.tensor_tensor(out=sm[:], in0=gh[:], in1=gt[:], op=ADD)
                    tp = epp.tile([64, 128], F32, tag=f"tp{half}", space="PSUM")
                    nc.tensor.transpose(out=tp[:], in_=sm[:], identity=idt[:])
                    nc.vector.tensor_copy(out=hinit[64:128, 128 * half : 128 * half + 128], in_=tp[:])

            # ================= decoder (B=256) =================
            with (
                tc.tile_pool(name="dwork", bufs=3) as dp_pool,
                tc.tile_pool(name="dpsum", bufs=2, space="PSUM") as dpp,
                tc.tile_pool(name="dpsum1", bufs=1, space="PSUM") as dpp1,
            ):
                nc.vector.memset(h2g[:], 0.0)
                nc.sync.dma_start(out=res[:], in_=dec_init_i[:])
                ones256 = ones[:, 0:BD]
                for t in range(T):
                    g1p = dpp.tile([128, 1024], F32, tag="g1p", space="PSUM")
                    nc.tensor.matmul(g1p[:, 0:256], lhsT=g1whh[:, 0:128], rhs=hinit[:], start=True, stop=False)
                    nc.tensor.matmul(g1p[:, 0:256], lhsT=g1x[:, 0:128], rhs=res[:], start=False, stop=True)
                    nc.tensor.matmul(g1p[:, 256:512], lhsT=g1whh[:, 128:256], rhs=hinit[:], start=True, stop=False)
                    nc.tensor.matmul(g1p[:, 256:512], lhsT=g1x[:, 128:256], rhs=res[:], start=False, stop=True)
                    nc.tensor.matmul(g1p[:, 512:768], lhsT=g1x[:, 256:384], rhs=res[:], start=True, stop=True)
                    nc.tensor.matmul(g1p[:, 768:1024], lhsT=g1whh[:, 256:384], rhs=hinit[:], start=True, stop=False)
                    nc.tensor.matmul(g1p[:, 768:1024], lhsT=g1bhhn[:], rhs=ones256, start=False, stop=True)
                    rz_sb = dp_pool.tile([128, 512], F32, tag="rz_sb")
                    nc.scalar.activation(rz_sb[:], g1p[:, 0:512], SIG)
                    tt = dp_pool.tile([128, 256], F32, tag="tt")
                    nc.vector.tensor_tensor(out=tt[:], in0=rz_sb[:, 0:256], in1=g1p[:, 768:1024], op=MUL)
                    nc.vector.tensor_tensor(out=tt[:], in0=tt[:], in1=g1p[:, 512:768], op=ADD)
                    n_sb = dp_pool.tile([128, 256], F32, tag="n_sb")
                    nc.scalar.activation(n_sb[:], tt[:], TANH)
                    dtl = dp_pool.tile([128, 256], F32, tag="dtl")
                    nc.gpsimd.tensor_tensor(out=dtl[:], in0=hinit[:], in1=n_sb[:], op=SUB)
                    nc.gpsimd.tensor_tensor(out=dtl[:], in0=rz_sb[:, 256:512], in1=dtl[:], op=MUL)
                    nc.gpsimd.tensor_tensor(out=hinit[:], in0=n_sb[:], in1=dtl[:], op=ADD)
                    # GRU2
                    g2p = dpp1.tile([50, 1024], F32, tag="g2p", space="PSUM")
                    nc.tensor.matmul(g2p[:, 0:256], lhsT=g2wx[:, 0:50], rhs=hinit[:], start=True, stop=False)
                    nc.tensor.matmul(g2p[:, 0:256], lhsT=g2whh[:, 0:50], rhs=h2g[:], start=False, stop=False)
                    nc.tensor.matmul(g2p[:, 0:256], lhsT=g2brz[:, 0:50], rhs=ones256, start=False, stop=True)
                    nc.tensor.matmul(g2p[:, 256:512], lhsT=g2wx[:, 50:100], rhs=hinit[:], start=True, stop=False)
                    nc.tensor.matmul(g2p[:, 256:512], lhsT=g2whh[:, 50:100], rhs=h2g[:], start=False, stop=False)
                    nc.tensor.matmul(g2p[:, 256:512], lhsT=g2brz[:, 50:100], rhs=ones256, start=False, stop=True)
                    nc.tensor.matmul(g2p[:, 512:768], lhsT=g2wx[:, 100:150], rhs=hinit[:], start=True, stop=True)
                    nc.tensor.matmul(g2p[:, 768:1024], lhsT=g2whh[:, 100:150], rhs=h2g[:], start=True, stop=False)
                    nc.tensor.matmul(g2p[:, 768:1024], lhsT=g2bhhn[:], rhs=ones256, start=False, stop=True)
                    rz2 = dp_pool.tile([50, 512], F32, tag="rz2")
                    nc.scalar.activation(rz2[:], g2p[:, 0:512], SIG)
                    t2t = dp_pool.tile([50, 256], F32, tag="t2t")
                    nc.vector.tensor_tensor(out=t2t[:], in0=rz2[:, 0:256], in1=g2p[:, 768:1024], op=MUL)
                    nc.vector.tensor_tensor(out=t2t[:], in0=t2t[:], in1=g2p[:, 512:768], op=ADD)
                    n2 = dp_pool.tile([50, 256], F32, tag="n2")
                    nc.scalar.activation(n2[:], t2t[:], TANH, bias=g2bn[:])
                    d2 = dp_pool.tile([50, 256], F32, tag="d2")
                    nc.vector.tensor_tensor(out=d2[:], in0=h2g[:], in1=n2[:], op=SUB)
                    nc.vector.tensor_tensor(out=d2[:], in0=rz2[:, 256:512], in1=d2[:], op=MUL)
                    nc.vector.tensor_tensor(out=h2g[:], in0=n2[:], in1=d2[:], op=ADD)
                    # dec fc (replicated rows)
                    dcp = dpp.tile([128, 256], F32, tag="dcp", space="PSUM")
                    nc.tensor.matmul(dcp[:], lhsT=decw[:], rhs=h2g[:], start=True, stop=True)
                    nc.scalar.activation(res[0:1, :], dcp[0:1, :], SIG, bias=decb[0:1, :])
                    k = t % 8
                    if k == 0:
                        oblk = dp_pool.tile([1, 8 * BD], F32, tag="oblk")
                    nc.scalar.activation(
                        oblk[0:1, BD * k : BD * k + BD], dcp[0:1, :], SIG,
                        bias=decb[0:1, :])
                    if k == 7:
                        nc.sync.dma_start(out=out_staged[t // 8], in_=oblk[:])

    nc.finalize()
    return nc


def _prep_inputs(inputs):
    inp = {k: np.ascontiguousarray(np.asarray(v)) for k, v in inputs.items()}
    edge = inp["edge_data"][:, :, 0].astype(np.float32)  # (2048, 200)
    node = inp["node_data"].astype(np.int32)
    onesBE = np.ones((1, BE), np.float32)

    def lstm1_dir(d):  # d in 'fb'
        whhT = inp[f"l1_whh_{d}"].T.astype(np.float32)  # (128, 512)
        xaug = np.stack([inp[f"l1_wih_{d}"][:, 0], inp[f"l1_b_{d}"]]).astype(np.float32)
        return whhT, xaug

    wf, xf_ = lstm1_dir("f")
    wb, xb_ = lstm1_dir("b")

    g1_bias = np.concatenate(
        [(inp["g1_bih"] + inp["g1_bhh"])[0:256], inp["g1_bih"][256:384]])
    shared = dict(
        fc1_wT=np.ascontiguousarray(inp["fc1_w"].T.reshape(2, 128, 256)),
        fc1_bias=np.ascontiguousarray(inp["fc1_b"].reshape(2, 128).T),
        fc2_wT=np.ascontiguousarray(inp["fc2_w"].T.reshape(2, 128, 64)),
        fc2_bias=inp["fc2_b"][:, None].astype(np.float32),
        g1_whhT=np.ascontiguousarray(inp["g1_whh"].T),
        g1_xaug=np.ascontiguousarray(np.stack([inp["g1_wih"][:, 0], g1_bias])),
        g1_bhhn=np.ascontiguousarray(inp["g1_bhh"][None, 256:384]),
        g2_wxT=np.ascontiguousarray(inp["g2_wih"].T),
        g2_whhT=np.ascontiguousarray(inp["g2_whh"].T),
        g2_brz=np.ascontiguousarray((inp["g2_bih"] + inp["g2_bhh"])[None, 0:100]),
        g2_bn=np.ascontiguousarray(inp["g2_bih"][100:150, None]),
        g2_bhhn=np.ascontiguousarray(inp["g2_bhh"][None, 100:150]),
        dec_wTr=np.ascontiguousarray(np.repeat(inp["dec_w"].T, 128, axis=1)),
        dec_br=np.ascontiguousarray(np.repeat(inp["dec_b"][:, None], 128, axis=0)),
        ones_in=onesBE,
        ident=(0.5 * np.eye(128)).astype(np.float32),
        emb=inp["emb"].astype(np.float32),
    )
    in_maps = []
    for c in range(NC):
        cp = c % 4
        J = slice(512 * cp, 512 * cp + 512)
        I = slice(256 * c, 256 * c + 256)
        rev = c >= 4
        eT = edge[J].T  # (200, 512)
        if rev:
            eT = eT[::-1]
        edge_src = np.ascontiguousarray(
            np.stack([eT, np.broadcast_to(onesBE, (T, BE))], axis=1))
        d2 = "b" if rev else "f"  # LSTM2 direction this core needs
        m = dict(shared)
        m["edge_src"] = edge_src
        m["l1_whhT"] = np.ascontiguousarray(np.stack([wb, wf] if rev else [wf, wb]))
        m["l1_xaug"] = np.ascontiguousarray(np.stack([xb_, xf_] if rev else [xf_, xb_]))
        m["l2_wihT"] = np.ascontiguousarray(
            inp[f"l2_wih_{d2}"].T.reshape(2, 128, 512).astype(np.float32))
        m["l2_whhT"] = np.ascontiguousarray(inp[f"l2_whh_{d2}"].T)
        m["l2_bias"] = np.ascontiguousarray(inp[f"l2_b_{d2}"].reshape(4, 128).T)
        m["node_idx"] = np.ascontiguousarray(node[I])
        m["dec_init"] = np.ascontiguousarray(
            np.stack([edge[I, -1], np.ones(BD, np.float32)]))
        in_maps.append(m)
    return in_maps


def run_device(inputs, trace=False):
    if "nc" not in _CACHE:
        _CACHE["nc"] = _build_program()
    nc = _CACHE["nc"]
    in_maps = _prep_inputs(inputs)
    br = run_bass_kernel_spmd(nc, in_maps, list(range(NC)), trace=trace)
    out = np.zeros((2048, T, 1), np.float32)
    for c in range(NC):
        staged = br.results[c]["out_staged"].reshape(25, 8, BD)  # (blk, k, b)
        out[256 * c : 256 * c + 256, :, 0] = staged.reshape(T, BD).T
    return out, br


def kernel(**inputs) -> np.ndarray:
    out, _ = run_device(inputs, trace=False)
    return out



# revision 3
# speedup vs baseline: 16.3264x; 16.3264x over previous
# Trainium2 Bass kernel for nn_Graph_AutoEncoder (BiLSTM encoder + GRU decoder).
#
# Sharding: decoder rows i in [256c, 256c+256) per core c. Each core encodes the
# 512 batch rows j = 2i, 2i+1 its decoder slice needs (LSTM1 both dirs at B=512,
# LSTM2 one direction). Cores 4-7 need the *backward* LSTM2 direction; they get
# time-reversed edge sequences and f/b-swapped LSTM1 weights via their input map,
# so the compiled program is identical on all 8 cores (SPMD, no collectives).
#
# Host/transfer strategy (the dominant cost on axon-tunneled cores): the PJRT
# executable is compiled once and cached; all weight-derived tensors are kept
# device-resident across calls (re-uploaded only when the weight bytes change);
# the 50k x 64 embedding lookup happens on host so only the gathered (64, 256)
# slice per core is shipped. Per call only edge data + gathered node embeddings
# + decoder init (~4 MB total) cross the tunnel.
#
# Layout: feature-on-partition. Gates are computed as W @ h matmuls into PSUM
# (lhsT = W^T with K on partitions); biases are folded into ScalarE activation
# bias operands (LSTM1/LSTM2) or bias-row matmuls against an on-chip ones tile
# (decoder GRUs).
import hashlib

import numpy as np

import jax
import jax.numpy as jnp
from jax.sharding import Mesh, NamedSharding, PartitionSpec

import concourse.bass as bass
import concourse.bass2jax as b2j
import concourse.mybir as mybir
import concourse.tile as tile
from concourse import bacc

F32 = mybir.dt.float32
SIG = mybir.ActivationFunctionType.Sigmoid
TANH = mybir.ActivationFunctionType.Tanh
MUL = mybir.AluOpType.mult
ADD = mybir.AluOpType.add
SUB = mybir.AluOpType.subtract

T = 200
NC = 8
BE = 512  # encoder batch per core
BD = 256  # decoder batch per core

# Inputs that change per call; everything else is weight-derived and cached
# on-device between calls.
DYNAMIC = ("edge_src", "node_embT", "dec_init")

_CACHE = {}


def _build_program():
    nc = bacc.Bacc("TRN2", target_bir_lowering=False, debug=False, num_devices=NC)

    def din(name, shape, d=F32):
        return nc.dram_tensor(name, shape, d, kind="ExternalInput").ap()

    edge_src = din("edge_src", [T, BE])
    node_embT_i = din("node_embT", [64, BD])
    dec_init_i = din("dec_init", [2, BD])
    l1_whhT = din("l1_whhT", [2, 128, 512])
    l1_wxT_i = din("l1_wxT", [1, 1024])
    l1_bias_i = din("l1_bias", [128, 8])
    l2_wihT = din("l2_wihT", [2, 128, 512])
    l2_whhT_i = din("l2_whhT", [128, 512])
    l2_bias_i = din("l2_bias", [128, 4])
    fc1_wT = din("fc1_wT", [2, 128, 256])
    fc1_bias_i = din("fc1_bias", [128, 2])
    fc2_wT = din("fc2_wT", [2, 128, 64])
    fc2_bias_i = din("fc2_bias", [64, 1])
    g1_whhT_i = din("g1_whhT", [128, 384])
    g1_xaug_i = din("g1_xaug", [2, 384])
    g1_bhhn_i = din("g1_bhhn", [1, 128])
    g2_wxT_i = din("g2_wxT", [128, 150])
    g2_whhT_i = din("g2_whhT", [50, 150])
    g2_brz_i = din("g2_brz", [1, 100])
    g2_bn_i = din("g2_bn", [50, 1])
    g2_bhhn_i = din("g2_bhhn", [1, 50])
    dec_wTr_i = din("dec_wTr", [50, 128])
    dec_br_i = din("dec_br", [128, 1])

    out_staged = nc.dram_tensor("out_staged", [25, 8 * BD], F32, kind="ExternalOutput").ap()
    h1_buf = nc.dram_tensor("h1_buf", [2, T, 128, BE], F32).ap()

    with tile.TileContext(nc) as tc:
        with (
            tc.tile_pool(name="wpool", bufs=1) as wp,
            tc.tile_pool(name="spool", bufs=1) as sp,
        ):
            # ---- persistent weights ----
            l1w = wp.tile([128, 1024], F32)
            nc.sync.dma_start(out=l1w[:, 0:512], in_=l1_whhT[0])
            nc.sync.dma_start(out=l1w[:, 512:1024], in_=l1_whhT[1])
            l1wx = wp.tile([1, 1024], F32)
            nc.sync.dma_start(out=l1wx[:], in_=l1_wxT_i[:])
            l1b = wp.tile([128, 8], F32)
            nc.sync.dma_start(out=l1b[:], in_=l1_bias_i[:])
            l2wih = wp.tile([128, 1024], F32)
            nc.sync.dma_start(out=l2wih[:, 0:512], in_=l2_wihT[0])
            nc.sync.dma_start(out=l2wih[:, 512:1024], in_=l2_wihT[1])
            l2whh = wp.tile([128, 512], F32)
            nc.sync.dma_start(out=l2whh[:], in_=l2_whhT_i[:])
            l2b = wp.tile([128, 4], F32)
            nc.sync.dma_start(out=l2b[:], in_=l2_bias_i[:])
            fc1w = wp.tile([128, 512], F32)
            nc.sync.dma_start(out=fc1w[:, 0:256], in_=fc1_wT[0])
            nc.sync.dma_start(out=fc1w[:, 256:512], in_=fc1_wT[1])
            fc1b = wp.tile([128, 2], F32)
            nc.sync.dma_start(out=fc1b[:], in_=fc1_bias_i[:])
            fc2w = wp.tile([128, 128], F32)
            nc.sync.dma_start(out=fc2w[:, 0:64], in_=fc2_wT[0])
            nc.sync.dma_start(out=fc2w[:, 64:128], in_=fc2_wT[1])
            fc2b = wp.tile([64, 1], F32)
            nc.sync.dma_start(out=fc2b[:], in_=fc2_bias_i[:])
            g1whh = wp.tile([128, 384], F32)
            nc.sync.dma_start(out=g1whh[:], in_=g1_whhT_i[:])
            g1x = wp.tile([2, 384], F32)
            nc.sync.dma_start(out=g1x[:], in_=g1_xaug_i[:])
            g1bhhn = wp.tile([1, 128], F32)
            nc.sync.dma_start(out=g1bhhn[:], in_=g1_bhhn_i[:])
            g2wx = wp.tile([128, 150], F32)
            nc.sync.dma_start(out=g2wx[:], in_=g2_wxT_i[:])
            g2whh = wp.tile([50, 150], F32)
            nc.sync.dma_start(out=g2whh[:], in_=g2_whhT_i[:])
            g2brz = wp.tile([1, 100], F32)
            nc.sync.dma_start(out=g2brz[:], in_=g2_brz_i[:])
            g2bn = wp.tile([50, 1], F32)
            nc.sync.dma_start(out=g2bn[:], in_=g2_bn_i[:])
            g2bhhn = wp.tile([1, 50], F32)
            nc.sync.dma_start(out=g2bhhn[:], in_=g2_bhhn_i[:])
            decw = wp.tile([50, 128], F32)
            nc.sync.dma_start(out=decw[:], in_=dec_wTr_i[:])
            decb = wp.tile([128, 1], F32)
            nc.sync.dma_start(out=decb[:], in_=dec_br_i[:])
            ones = wp.tile([1, BE], F32)
            nc.vector.memset(ones[:], 1.0)

            # ---- persistent state ----
            hn1 = sp.tile([128, BE], F32)
            hn2cap = sp.tile([128, BE], F32)
            hinit = sp.tile([128, BD], F32)
            h2g = sp.tile([50, BD], F32)
            res = sp.tile([2, BD], F32)

            # ================= LSTM1 (both dirs, B=512) =================
            with (
                tc.tile_pool(name="l1ring", bufs=8) as rp,
                tc.tile_pool(name="l1hring", bufs=4) as hp,
                tc.tile_pool(name="l1work", bufs=3) as kp,
                tc.tile_pool(name="l1state", bufs=1) as lsp,
                tc.tile_pool(name="l1psum", bufs=1, space="PSUM") as pp,
            ):
                c1 = lsp.tile([128, 1024], F32)
                nc.vector.memset(c1[:], 0.0)
                h_prev = hp.tile([128, 1024], F32, tag="h1o")
                nc.vector.memset(h_prev[:], 0.0)
                for s in range(T):
                    erf = rp.tile([1, BE], F32, tag="erf")
                    nc.sync.dma_start(out=erf[:], in_=edge_src[s : s + 1])
                    erb = rp.tile([1, BE], F32, tag="erb")
                    nc.sync.dma_start(out=erb[:], in_=edge_src[T - 1 - s : T - s])
                    sigp = pp.tile([128, 3072], F32, tag="sigp", space="PSUM")
                    gp = pp.tile([128, 1024], F32, tag="gp", space="PSUM")
                    for d in (0, 1):
                        hs_d = h_prev[:, 512 * d : 512 * d + 512]
                        er_d = erf if d == 0 else erb
                        for gi, g in ((0, 0), (1, 1), (2, 3), (3, 2)):
                            if g == 2:  # tanh gate
                                dst = gp[:, 512 * d : 512 * d + 512]
                            else:
                                col = (0, 1, None, 2)[g]
                                dst = sigp[:, 1024 * col + 512 * d : 1024 * col + 512 * d + 512]
                            nc.tensor.matmul(
                                dst, lhsT=l1w[:, 512 * d + 128 * g : 512 * d + 128 * g + 128],
                                rhs=hs_d, start=True, stop=False)
                            nc.tensor.matmul(
                                dst, lhsT=l1wx[:, 512 * d + 128 * g : 512 * d + 128 * g + 128],
                                rhs=er_d[:], start=False, stop=True)
                    sig_sb = kp.tile([128, 3072], F32, tag="sig_sb")
                    for d in (0, 1):
                        for g in (0, 1, 3):
                            col = (0, 1, None, 2)[g]
                            off = 1024 * col + 512 * d
                            nc.scalar.activation(
                                sig_sb[:, off : off + 512], sigp[:, off : off + 512],
                                SIG, bias=l1b[:, 4 * d + g : 4 * d + g + 1])
                    g_sb = kp.tile([128, 1024], F32, tag="g_sb")
                    for d in (0, 1):
                        nc.scalar.activation(
                            g_sb[:, 512 * d : 512 * d + 512], gp[:, 512 * d : 512 * d + 512],
                            TANH, bias=l1b[:, 4 * d + 2 : 4 * d + 3])
                    t1 = kp.tile([128, 1024], F32, tag="t1")
                    nc.vector.tensor_tensor(out=t1[:], in0=sig_sb[:, 0:1024], in1=g_sb[:], op=MUL)
                    nc.vector.tensor_tensor(out=c1[:], in0=sig_sb[:, 1024:2048], in1=c1[:], op=MUL)
                    nc.vector.tensor_tensor(out=c1[:], in0=c1[:], in1=t1[:], op=ADD)
                    tc_sb = kp.tile([128, 1024], F32, tag="tc_sb")
                    nc.scalar.activation(tc_sb[:], c1[:], TANH)
                    h_cur = hp.tile([128, 1024], F32, tag="h1o")
                    nc.gpsimd.tensor_tensor(out=h_cur[:], in0=sig_sb[:, 2048:3072], in1=tc_sb[:], op=MUL)
                    nc.sync.dma_start(out=h1_buf[0, s], in_=h_cur[:, 0:512])
                    nc.sync.dma_start(out=h1_buf[1, T - 1 - s], in_=h_cur[:, 512:1024])
                    h_prev = h_cur
                nc.vector.tensor_copy(out=hn1[:], in_=h_prev[:, 0:512])

            # ================= LSTM2 (one dir, B=512) =================
            with (
                tc.tile_pool(name="l2ring", bufs=8) as rp2,
                tc.tile_pool(name="l2work", bufs=3) as kp2,
                tc.tile_pool(name="l2state", bufs=1) as lsp2,
                tc.tile_pool(name="l2psum", bufs=2, space="PSUM") as pp2,
            ):
                c2 = lsp2.tile([128, 512], F32)
                nc.vector.memset(c2[:], 0.0)
                h2p = lsp2.tile([128, 512], F32)
                nc.vector.memset(h2p[:], 0.0)
                h2n = lsp2.tile([128, 512], F32)
                for s in range(T):
                    xf = rp2.tile([128, 512], F32, tag="xf")
                    nc.sync.dma_start(out=xf[:], in_=h1_buf[0, s])
                    xb = rp2.tile([128, 512], F32, tag="xb")
                    nc.sync.dma_start(out=xb[:], in_=h1_buf[1, s])
                    sp2t = pp2.tile([128, 1536], F32, tag="sp2", space="PSUM")
                    gp2 = pp2.tile([128, 512], F32, tag="gp2", space="PSUM")
                    for g, dst_info in ((0, (sp2t, 0)), (1, (sp2t, 512)), (3, (sp2t, 1024)), (2, (gp2, 0))):
                        dtile, off = dst_info
                        dst = dtile[:, off : off + 512]
                        nc.tensor.matmul(dst, lhsT=l2wih[:, 128 * g : 128 * g + 128], rhs=xf[:], start=True, stop=False)
                        nc.tensor.matmul(dst, lhsT=l2wih[:, 512 + 128 * g : 512 + 128 * g + 128], rhs=xb[:], start=False, stop=False)
                        nc.tensor.matmul(dst, lhsT=l2whh[:, 128 * g : 128 * g + 128], rhs=h2p[:], start=False, stop=True)
                    sb2 = kp2.tile([128, 1536], F32, tag="sb2")
                    nc.scalar.activation(sb2[:, 0:512], sp2t[:, 0:512], SIG, bias=l2b[:, 0:1])
                    nc.scalar.activation(sb2[:, 512:1024], sp2t[:, 512:1024], SIG, bias=l2b[:, 1:2])
                    nc.scalar.activation(sb2[:, 1024:1536], sp2t[:, 1024:1536], SIG, bias=l2b[:, 3:4])
                    g2sb = kp2.tile([128, 512], F32, tag="g2sb")
                    nc.scalar.activation(g2sb[:], gp2[:], TANH, bias=l2b[:, 2:3])
                    t2 = kp2.tile([128, 512], F32, tag="t2")
                    nc.vector.tensor_tensor(out=t2[:], in0=sb2[:, 0:512], in1=g2sb[:], op=MUL)
                    nc.vector.tensor_tensor(out=c2[:], in0=sb2[:, 512:1024], in1=c2[:], op=MUL)
                    nc.vector.tensor_tensor(out=c2[:], in0=c2[:], in1=t2[:], op=ADD)
                    tc2 = kp2.tile([128, 512], F32, tag="tc2")
                    nc.scalar.activation(tc2[:], c2[:], TANH)
                    dst_h = hn2cap if s == T - 1 else (h2n if s % 2 == 0 else h2p)
                    nc.gpsimd.tensor_tensor(out=dst_h[:], in0=sb2[:, 1024:1536], in1=tc2[:], op=MUL)
                    h2p, h2n = dst_h, (h2p if s % 2 == 0 else h2n)

            # ================= encoder tail =================
            with (
                tc.tile_pool(name="etwork", bufs=1) as ep,
                tc.tile_pool(name="etpsum", bufs=1, space="PSUM") as epp,
            ):
                hnsum = ep.tile([128, BE], F32)
                nc.vector.tensor_tensor(out=hnsum[:], in0=hn1[:], in1=hn2cap[:], op=ADD)
                X = ep.tile([128, 512], F32)
                hv = hnsum[:].rearrange("p (k two) -> p two k", two=2)
                nc.vector.tensor_copy(out=X[:, 0:256], in_=hv[:, 0, :])
                nc.vector.tensor_copy(out=X[:, 256:512], in_=hv[:, 1, :])
                fc1p = epp.tile([128, 512], F32, tag="fc1p", space="PSUM")
                for m in (0, 1):
                    dst = fc1p[:, 256 * m : 256 * m + 256]
                    nc.tensor.matmul(dst, lhsT=fc1w[:, 128 * m : 128 * m + 128], rhs=X[:, 0:256], start=True, stop=False)
                    nc.tensor.matmul(dst, lhsT=fc1w[:, 256 + 128 * m : 256 + 128 * m + 128], rhs=X[:, 256:512], start=False, stop=True)
                Y = ep.tile([128, 512], F32)
                nc.scalar.activation(Y[:, 0:256], fc1p[:, 0:256], SIG, bias=fc1b[:, 0:1])
                nc.scalar.activation(Y[:, 256:512], fc1p[:, 256:512], SIG, bias=fc1b[:, 1:2])
                fc2p = epp.tile([64, 256], F32, tag="fc2p", space="PSUM")
                nc.tensor.matmul(fc2p[:], lhsT=fc2w[:, 0:64], rhs=Y[:, 0:256], start=True, stop=False)
                nc.tensor.matmul(fc2p[:], lhsT=fc2w[:, 64:128], rhs=Y[:, 256:512], start=False, stop=True)
                nc.scalar.activation(hinit[0:64, :], fc2p[:], SIG, bias=fc2b[:])
                # node embeddings are gathered host-side; just load the slice
                nc.sync.dma_start(out=hinit[64:128, :], in_=node_embT_i[:])

            # ================= decoder (B=256) =================
            with (
                tc.tile_pool(name="dwork", bufs=3) as dp_pool,
                tc.tile_pool(name="dpsum", bufs=2, space="PSUM") as dpp,
                tc.tile_pool(name="dpsum1", bufs=1, space="PSUM") as dpp1,
            ):
                nc.vector.memset(h2g[:], 0.0)
                nc.sync.dma_start(out=res[:], in_=dec_init_i[:])
                ones256 = ones[:, 0:BD]
                for t in range(T):
                    g1p = dpp.tile([128, 1024], F32, tag="g1p", space="PSUM")
                    nc.tensor.matmul(g1p[:, 0:256], lhsT=g1whh[:, 0:128], rhs=hinit[:], start=True, stop=False)
                    nc.tensor.matmul(g1p[:, 0:256], lhsT=g1x[:, 0:128], rhs=res[:], start=False, stop=True)
                    nc.tensor.matmul(g1p[:, 256:512], lhsT=g1whh[:, 128:256], rhs=hinit[:], start=True, stop=False)
                    nc.tensor.matmul(g1p[:, 256:512], lhsT=g1x[:, 128:256], rhs=res[:], start=False, stop=True)
                    nc.tensor.matmul(g1p[:, 512:768], lhsT=g1x[:, 256:384], rhs=res[:], start=True, stop=True)
                    nc.tensor.matmul(g1p[:, 768:1024], lhsT=g1whh[:, 256:384], rhs=hinit[:], start=True, stop=False)
                    nc.tensor.matmul(g1p[:, 768:1024], lhsT=g1bhhn[:], rhs=ones256, start=False, stop=True)
                    rz_sb = dp_pool.tile([128, 512], F32, tag="rz_sb")
                    nc.scalar.activation(rz_sb[:], g1p[:, 0:512], SIG)
                    tt = dp_pool.tile([128, 256], F32, tag="tt")
                    nc.vector.tensor_tensor(out=tt[:], in0=rz_sb[:, 0:256], in1=g1p[:, 768:1024], op=MUL)
                    nc.vector.tensor_tensor(out=tt[:], in0=tt[:], in1=g1p[:, 512:768], op=ADD)
                    n_sb = dp_pool.tile([128, 256], F32, tag="n_sb")
                    nc.scalar.activation(n_sb[:], tt[:], TANH)
                    dtl = dp_pool.tile([128, 256], F32, tag="dtl")
                    nc.gpsimd.tensor_tensor(out=dtl[:], in0=hinit[:], in1=n_sb[:], op=SUB)
                    nc.gpsimd.tensor_tensor(out=dtl[:], in0=rz_sb[:, 256:512], in1=dtl[:], op=MUL)
                    nc.gpsimd.tensor_tensor(out=hinit[:], in0=n_sb[:], in1=dtl[:], op=ADD)
                    # GRU2
                    g2p = dpp1.tile([50, 1024], F32, tag="g2p", space="PSUM")
                    nc.tensor.matmul(g2p[:, 0:256], lhsT=g2wx[:, 0:50], rhs=hinit[:], start=True, stop=False)
                    nc.tensor.matmul(g2p[:, 0:256], lhsT=g2whh[:, 0:50], rhs=h2g[:], start=False, stop=False)
                    nc.tensor.matmul(g2p[:, 0:256], lhsT=g2brz[:, 0:50], rhs=ones256, start=False, stop=True)
                    nc.tensor.matmul(g2p[:, 256:512], lhsT=g2wx[:, 50:100], rhs=hinit[:], start=True, stop=False)
                    nc.tensor.matmul(g2p[:, 256:512], lhsT=g2whh[:, 50:100], rhs=h2g[:], start=False, stop=False)
                    nc.tensor.matmul(g2p[:, 256:512], lhsT=g2brz[:, 50:100], rhs=ones256, start=False, stop=True)
                    nc.tensor.matmul(g2p[:, 512:768], lhsT=g2wx[:, 100:150], rhs=hinit[:], start=True, stop=True)
                    nc.tensor.matmul(g2p[:, 768:1024], lhsT=g2whh[:, 100:150], rhs=h2g[:], start=True, stop=False)
                    nc.tensor.matmul(g2p[:, 768:1024], lhsT=g2bhhn[:], rhs=ones256, start=False, stop=True)
                    rz2 = dp_pool.tile([50, 512], F32, tag="rz2")
                    nc.scalar.activation(rz2[:], g2p[:, 0:512], SIG)
                    t2t = dp_pool.tile([50, 256], F32, tag="t2t")
                    nc.vector.tensor_tensor(out=t2t[:], in0=rz2[:, 0:256], in1=g2p[:, 768:1024], op=MUL)
                    nc.vector.tensor_tensor(out=t2t[:], in0=t2t[:], in1=g2p[:, 512:768], op=ADD)
                    n2 = dp_pool.tile([50, 256], F32, tag="n2")
                    nc.scalar.activation(n2[:], t2t[:], TANH, bias=g2bn[:])
                    d2 = dp_pool.tile([50, 256], F32, tag="d2")
                    nc.vector.tensor_tensor(out=d2[:], in0=h2g[:], in1=n2[:], op=SUB)
                    nc.vector.tensor_tensor(out=d2[:], in0=rz2[:, 256:512], in1=d2[:], op=MUL)
                    nc.vector.tensor_tensor(out=h2g[:], in0=n2[:], in1=d2[:], op=ADD)
                    # dec fc (replicated rows)
                    dcp = dpp.tile([128, 256], F32, tag="dcp", space="PSUM")
                    nc.tensor.matmul(dcp[:], lhsT=decw[:], rhs=h2g[:], start=True, stop=True)
                    nc.scalar.activation(res[0:1, :], dcp[0:1, :], SIG, bias=decb[0:1, :])
                    k = t % 8
                    if k == 0:
                        oblk = dp_pool.tile([1, 8 * BD], F32, tag="oblk")
                    nc.scalar.activation(
                        oblk[0:1, BD * k : BD * k + BD], dcp[0:1, :], SIG,
                        bias=decb[0:1, :])
                    if k == 7:
                        nc.sync.dma_start(out=out_staged[t // 8], in_=oblk[:])

    nc.finalize()
    return nc


def _prep_static(inputs):
    """Per-core weight-derived arrays (everything except DYNAMIC tensors)."""
    inp = {k: np.asarray(v) for k, v in inputs.items()}

    def lstm1_dir(d):  # d in 'fb'
        whhT = np.ascontiguousarray(inp[f"l1_whh_{d}"].T.astype(np.float32))  # (128, 512)
        wx = inp[f"l1_wih_{d}"][:, 0].astype(np.float32)  # (512,)
        bias = inp[f"l1_b_{d}"].astype(np.float32).reshape(4, 128).T  # (128, 4)
        return whhT, wx, bias

    wf, wxf, bf = lstm1_dir("f")
    wb, wxb, bb = lstm1_dir("b")

    g1_bias = np.concatenate(
        [(inp["g1_bih"] + inp["g1_bhh"])[0:256], inp["g1_bih"][256:384]])
    shared = dict(
        fc1_wT=np.ascontiguousarray(inp["fc1_w"].T.reshape(2, 128, 256)),
        fc1_bias=np.ascontiguousarray(inp["fc1_b"].reshape(2, 128).T),
        fc2_wT=np.ascontiguousarray(inp["fc2_w"].T.reshape(2, 128, 64)),
        fc2_bias=inp["fc2_b"][:, None].astype(np.float32),
        g1_whhT=np.ascontiguousarray(inp["g1_whh"].T),
        g1_xaug=np.ascontiguousarray(np.stack([inp["g1_wih"][:, 0], g1_bias])),
        g1_bhhn=np.ascontiguousarray(inp["g1_bhh"][None, 256:384]),
        g2_wxT=np.ascontiguousarray(inp["g2_wih"].T),
        g2_whhT=np.ascontiguousarray(inp["g2_whh"].T),
        g2_brz=np.ascontiguousarray((inp["g2_bih"] + inp["g2_bhh"])[None, 0:100]),
        g2_bn=np.ascontiguousarray(inp["g2_bih"][100:150, None]),
        g2_bhhn=np.ascontiguousarray(inp["g2_bhh"][None, 100:150]),
        dec_wTr=np.ascontiguousarray(np.repeat(inp["dec_w"].T, 128, axis=1)),
        dec_br=np.ascontiguousarray(np.repeat(inp["dec_b"][:, None], 128, axis=0)),
    )
    maps = []
    for c in range(NC):
        rev = c >= 4
        d2 = "b" if rev else "f"  # LSTM2 direction this core needs
        m = dict(shared)
        m["l1_whhT"] = np.ascontiguousarray(np.stack([wb, wf] if rev else [wf, wb]))
        m["l1_wxT"] = np.ascontiguousarray(
            np.concatenate([wxb, wxf] if rev else [wxf, wxb])[None, :])
        m["l1_bias"] = np.ascontiguousarray(
            np.concatenate([bb, bf] if rev else [bf, bb], axis=1))
        m["l2_wihT"] = np.ascontiguousarray(
            inp[f"l2_wih_{d2}"].T.reshape(2, 128, 512).astype(np.float32))
        m["l2_whhT"] = np.ascontiguousarray(inp[f"l2_whh_{d2}"].T)
        m["l2_bias"] = np.ascontiguousarray(inp[f"l2_b_{d2}"].reshape(4, 128).T)
        maps.append(m)
    return maps


def _prep_dynamic(inputs):
    """Per-core arrays that change every call: edge seqs, gathered node embs."""
    edge = np.asarray(inputs["edge_data"])[:, :, 0].astype(np.float32)  # (2048, 200)
    node = np.asarray(inputs["node_data"]).astype(np.int64)
    emb = np.asarray(inputs["emb"]).astype(np.float32)
    node_emb = 0.5 * (emb[node[:, 0]] + emb[node[:, 1]])  # (2048, 64)
    maps = []
    for c in range(NC):
        cp = c % 4
        J = slice(512 * cp, 512 * cp + 512)
        I = slice(256 * c, 256 * c + 256)
        eT = edge[J].T  # (200, 512)
        if c >= 4:
            eT = eT[::-1]
        maps.append(dict(
            edge_src=np.ascontiguousarray(eT),
            node_embT=np.ascontiguousarray(node_emb[I].T),
            dec_init=np.ascontiguousarray(
                np.stack([edge[I, -1], np.ones(BD, np.float32)])),
        ))
    return maps


def _weights_key(inputs):
    h = hashlib.sha1()
    for k in sorted(inputs):
        if k in ("node_data", "edge_data", "emb"):
            continue
        a = np.asarray(inputs[k])
        h.update(k.encode())
        h.update(a.tobytes())
    return h.hexdigest()


def _get_state():
    if "state" in _CACHE:
        return _CACHE["state"]
    nc = _build_program()
    b2j.install_neuronx_cc_hook()

    partition_name = nc.partition_id_tensor.name if nc.partition_id_tensor else None
    in_names, out_names, out_avals = [], [], []
    for alloc in nc.m.functions[0].allocations:
        if not isinstance(alloc, mybir.MemoryLocationSet):
            continue
        name = alloc.memorylocations[0].name
        if alloc.kind == "ExternalInput":
            if name != partition_name:
                in_names.append(name)
        elif alloc.kind == "ExternalOutput":
            out_names.append(name)
            out_avals.append(jax.core.ShapedArray(
                tuple(alloc.tensor_shape), mybir.dt.np(alloc.dtype)))
    in_names_all = list(in_names) + ([partition_name] if partition_name else [])

    def _body(*args):
        operands = list(args)
        if partition_name is not None:
            operands.append(b2j.partition_id_tensor())
        outs = b2j._bass_exec_p.bind(
            *operands, out_avals=tuple(out_avals), in_names=tuple(in_names_all),
            out_names=tuple(out_names), lowering_input_output_aliases=(),
            sim_require_finite=True, sim_require_nnan=True, nc=nc)
        return tuple(outs)

    devices = jax.devices()[:NC]
    mesh = Mesh(np.asarray(devices), ("core",))
    jitted = jax.jit(
        jax.shard_map(_body, mesh=mesh,
                      in_specs=(PartitionSpec("core"),) * len(in_names),
                      out_specs=(PartitionSpec("core"),) * len(out_names),
                      check_vma=False),
        keep_unused=True)

    # shapes/dtypes of the global (concat over cores) inputs, for lowering
    name_to_alloc = {}
    for alloc in nc.m.functions[0].allocations:
        if isinstance(alloc, mybir.MemoryLocationSet) and alloc.kind == "ExternalInput":
            name_to_alloc[alloc.memorylocations[0].name] = alloc
    sds = [
        jax.ShapeDtypeStruct(
            (NC * name_to_alloc[n].tensor_shape[0], *name_to_alloc[n].tensor_shape[1:]),
            mybir.dt.np(name_to_alloc[n].dtype))
        for n in in_names
    ]
    compiled = jitted.lower(*sds).compile()

    state = dict(
        compiled=compiled, in_names=in_names, out_names=out_names,
        sharding=NamedSharding(mesh, PartitionSpec("core")),
        static_key=None, static_dev={},
    )
    _CACHE["state"] = state
    return state


def _concat(maps, name):
    return np.concatenate([np.asarray(m[name]) for m in maps], axis=0)


def run_device(inputs, trace=False):
    st = _get_state()
    key = _weights_key(inputs)
    if st["static_key"] != key:
        smaps = _prep_static(inputs)
        dev = {}
        for n in smaps[0]:
            dev[n] = jax.device_put(_concat(smaps, n), st["sharding"])
        jax.block_until_ready(list(dev.values()))
        st["static_dev"] = dev
        st["static_key"] = key
    dmaps = _prep_dynamic(inputs)
    args = []
    for n in st["in_names"]:
        if n in DYNAMIC:
            args.append(_concat(dmaps, n))
        else:
            args.append(st["static_dev"][n])
    out_arrs = st["compiled"](*args)
    staged = np.asarray(out_arrs[0]).reshape(NC, 25, 8, BD)  # (core, blk, k, b)
    out = np.zeros((2048, T, 1), np.float32)
    for c in range(NC):
        out[256 * c : 256 * c + 256, :, 0] = staged[c].reshape(T, BD).T
    return out, None


def kernel(**inputs) -> np.ndarray:
    out, _ = run_device(inputs)
    return out


# revision 9
# speedup vs baseline: 28.3134x; 1.7342x over previous
# Trainium2 Bass kernel for nn_Graph_AutoEncoder (BiLSTM encoder + GRU decoder).
#
# Sharding: decoder rows i in [256c, 256c+256) per core c. Each core encodes the
# 512 batch rows j = 2i, 2i+1 its decoder slice needs (LSTM1 both dirs at B=512,
# LSTM2 one direction). Cores 4-7 need the *backward* LSTM2 direction; they get
# time-reversed edge sequences and f/b-swapped LSTM1 weights via their input map,
# so the compiled program is identical on all 8 cores (SPMD, no collectives).
#
# Host/transfer strategy (the dominant cost on axon-tunneled cores): the PJRT
# executable is compiled once and cached; all weight-derived tensors are kept
# device-resident across calls (re-uploaded only when the weight bytes change);
# the 50k x 64 embedding lookup happens on host so only the gathered (64, 256)
# slice per core is shipped. Per call only edge data + gathered node embeddings
# + decoder init (~4 MB total) cross the tunnel.
#
# Layout: feature-on-partition. Gates are computed as W @ h matmuls into PSUM
# (lhsT = W^T with K on partitions); biases are folded into ScalarE activation
# bias operands (LSTM1/LSTM2) or bias-row matmuls against an on-chip ones tile
# (decoder GRUs).
import hashlib

import numpy as np

import jax
import jax.numpy as jnp
from jax.sharding import Mesh, NamedSharding, PartitionSpec

import concourse.bass as bass
import concourse.bass2jax as b2j
import concourse.mybir as mybir
import concourse.tile as tile
from concourse import bacc

F32 = mybir.dt.float32
F16 = mybir.dt.float16
SIG = mybir.ActivationFunctionType.Sigmoid
TANH = mybir.ActivationFunctionType.Tanh
MUL = mybir.AluOpType.mult
ADD = mybir.AluOpType.add
SUB = mybir.AluOpType.subtract

T = 200
NC = 8
BE = 512  # encoder batch per core
BD = 256  # decoder batch per core

# Inputs that change per call; everything else is weight-derived and cached
# on-device between calls.
DYNAMIC = ("edge_src", "node_embT", "dec_init")

_CACHE = {}


def _build_program():
    nc = bacc.Bacc("TRN2", target_bir_lowering=False, debug=False, num_devices=NC)

    def din(name, shape, d=F32):
        return nc.dram_tensor(name, shape, d, kind="ExternalInput").ap()

    edge_src = din("edge_src", [T, BE], F16)
    node_embT_i = din("node_embT", [64, BD])
    dec_init_i = din("dec_init", [2, BD])
    l1_whhT = din("l1_whhT", [2, 128, 512])
    l1_wxT_i = din("l1_wxT", [1, 1024])
    l1_bias_i = din("l1_bias", [128, 8])
    l2_wihT = din("l2_wihT", [2, 128, 512])
    l2_whhT_i = din("l2_whhT", [128, 512])
    l2_bias_i = din("l2_bias", [128, 4])
    fc1_wT = din("fc1_wT", [2, 128, 256])
    fc1_bias_i = din("fc1_bias", [128, 2])
    fc2_wT = din("fc2_wT", [2, 128, 64])
    fc2_bias_i = din("fc2_bias", [64, 1])
    g1_whhT_i = din("g1_whhT", [128, 384])
    g1_xaug_i = din("g1_xaug", [2, 384])
    g1_bhhn_i = din("g1_bhhn", [1, 128])
    g2_wxT_i = din("g2_wxT", [128, 150])
    g2_whhT_i = din("g2_whhT", [50, 150])
    g2_brz_i = din("g2_brz", [1, 100])
    g2_bn_i = din("g2_bn", [50, 1])
    g2_bhhn_i = din("g2_bhhn", [1, 50])
    dec_wTr_i = din("dec_wTr", [50, 128])
    dec_br_i = din("dec_br", [128, 1])

    out_staged = nc.dram_tensor("out_staged", [25, 8 * BD], F16, kind="ExternalOutput").ap()
    h1_buf = nc.dram_tensor("h1_buf", [2, T, 128, BE], F32).ap()

    with tile.TileContext(nc) as tc:
        with (
            tc.tile_pool(name="wpool", bufs=1) as wp,
            tc.tile_pool(name="spool", bufs=1) as sp,
        ):
            # ---- persistent weights ----
            l1w = wp.tile([128, 1024], F32)
            nc.sync.dma_start(out=l1w[:, 0:512], in_=l1_whhT[0])
            nc.sync.dma_start(out=l1w[:, 512:1024], in_=l1_whhT[1])
            l1wx = wp.tile([1, 1024], F32)
            nc.sync.dma_start(out=l1wx[:], in_=l1_wxT_i[:])
            l1b = wp.tile([128, 8], F32)
            nc.sync.dma_start(out=l1b[:], in_=l1_bias_i[:])
            l2wih = wp.tile([128, 1024], F32)
            nc.sync.dma_start(out=l2wih[:, 0:512], in_=l2_wihT[0])
            nc.sync.dma_start(out=l2wih[:, 512:1024], in_=l2_wihT[1])
            l2whh = wp.tile([128, 512], F32)
            nc.sync.dma_start(out=l2whh[:], in_=l2_whhT_i[:])
            l2b = wp.tile([128, 4], F32)
            nc.sync.dma_start(out=l2b[:], in_=l2_bias_i[:])
            fc1w = wp.tile([128, 512], F32)
            nc.sync.dma_start(out=fc1w[:, 0:256], in_=fc1_wT[0])
            nc.sync.dma_start(out=fc1w[:, 256:512], in_=fc1_wT[1])
            fc1b = wp.tile([128, 2], F32)
            nc.sync.dma_start(out=fc1b[:], in_=fc1_bias_i[:])
            fc2w = wp.tile([128, 128], F32)
            nc.sync.dma_start(out=fc2w[:, 0:64], in_=fc2_wT[0])
            nc.sync.dma_start(out=fc2w[:, 64:128], in_=fc2_wT[1])
            fc2b = wp.tile([64, 1], F32)
            nc.sync.dma_start(out=fc2b[:], in_=fc2_bias_i[:])
            g1whh = wp.tile([128, 384], F32)
            nc.sync.dma_start(out=g1whh[:], in_=g1_whhT_i[:])
            g1x = wp.tile([2, 384], F32)
            nc.sync.dma_start(out=g1x[:], in_=g1_xaug_i[:])
            g1bhhn = wp.tile([1, 128], F32)
            nc.sync.dma_start(out=g1bhhn[:], in_=g1_bhhn_i[:])
            g2wx = wp.tile([128, 150], F32)
            nc.sync.dma_start(out=g2wx[:], in_=g2_wxT_i[:])
            g2whh = wp.tile([50, 150], F32)
            nc.sync.dma_start(out=g2whh[:], in_=g2_whhT_i[:])
            g2brz = wp.tile([1, 100], F32)
            nc.sync.dma_start(out=g2brz[:], in_=g2_brz_i[:])
            g2bn = wp.tile([50, 1], F32)
            nc.sync.dma_start(out=g2bn[:], in_=g2_bn_i[:])
            g2bhhn = wp.tile([1, 50], F32)
            nc.sync.dma_start(out=g2bhhn[:], in_=g2_bhhn_i[:])
            decw = wp.tile([50, 128], F32)
            nc.sync.dma_start(out=decw[:], in_=dec_wTr_i[:])
            decb = wp.tile([128, 1], F32)
            nc.sync.dma_start(out=decb[:], in_=dec_br_i[:])
            ones = wp.tile([1, BE], F32)
            nc.vector.memset(ones[:], 1.0)

            # ---- persistent state ----
            hn1 = sp.tile([128, BE], F32)
            hn2cap = sp.tile([128, BE], F32)
            hinit = sp.tile([128, BD], F32)
            h2g = sp.tile([50, BD], F32)
            res = sp.tile([2, BD], F32)

            # ================= LSTM1 (both dirs, B=512) =================
            with (
                tc.tile_pool(name="l1ring", bufs=8) as rp,
                tc.tile_pool(name="l1hring", bufs=4) as hp,
                tc.tile_pool(name="l1work", bufs=3) as kp,
                tc.tile_pool(name="l1state", bufs=1) as lsp,
                tc.tile_pool(name="l1psum", bufs=1, space="PSUM") as pp,
            ):
                c1 = lsp.tile([128, 1024], F32)
                nc.vector.memset(c1[:], 0.0)
                h_prev = hp.tile([128, 1024], F32, tag="h1o")
                nc.vector.memset(h_prev[:], 0.0)
                for s in range(T):
                    erf16 = rp.tile([1, BE], F16, tag="erf16")
                    nc.sync.dma_start(out=erf16[:], in_=edge_src[s : s + 1])
                    erf = rp.tile([1, BE], F32, tag="erf")
                    nc.vector.tensor_copy(out=erf[:], in_=erf16[:])
                    erb16 = rp.tile([1, BE], F16, tag="erb16")
                    nc.sync.dma_start(out=erb16[:], in_=edge_src[T - 1 - s : T - s])
                    erb = rp.tile([1, BE], F32, tag="erb")
                    nc.vector.tensor_copy(out=erb[:], in_=erb16[:])
                    sigp = pp.tile([128, 3072], F32, tag="sigp", space="PSUM")
                    gp = pp.tile([128, 1024], F32, tag="gp", space="PSUM")
                    for d in (0, 1):
                        hs_d = h_prev[:, 512 * d : 512 * d + 512]
                        er_d = erf if d == 0 else erb
                        for gi, g in ((0, 0), (1, 1), (2, 3), (3, 2)):
                            if g == 2:  # tanh gate
                                dst = gp[:, 512 * d : 512 * d + 512]
                            else:
                                col = (0, 1, None, 2)[g]
                                dst = sigp[:, 1024 * col + 512 * d : 1024 * col + 512 * d + 512]
                            nc.tensor.matmul(
                                dst, lhsT=l1w[:, 512 * d + 128 * g : 512 * d + 128 * g + 128],
                                rhs=hs_d, start=True, stop=False)
                            nc.tensor.matmul(
                                dst, lhsT=l1wx[:, 512 * d + 128 * g : 512 * d + 128 * g + 128],
                                rhs=er_d[:], start=False, stop=True)
                    sig_sb = kp.tile([128, 3072], F32, tag="sig_sb")
                    for d in (0, 1):
                        for g in (0, 1, 3):
                            col = (0, 1, None, 2)[g]
                            off = 1024 * col + 512 * d
                            nc.scalar.activation(
                                sig_sb[:, off : off + 512], sigp[:, off : off + 512],
                                SIG, bias=l1b[:, 4 * d + g : 4 * d + g + 1])
                    g_sb = kp.tile([128, 1024], F32, tag="g_sb")
                    for d in (0, 1):
                        nc.scalar.activation(
                            g_sb[:, 512 * d : 512 * d + 512], gp[:, 512 * d : 512 * d + 512],
                            TANH, bias=l1b[:, 4 * d + 2 : 4 * d + 3])
                    t1 = kp.tile([128, 1024], F32, tag="t1")
                    nc.vector.tensor_tensor(out=t1[:], in0=sig_sb[:, 0:1024], in1=g_sb[:], op=MUL)
                    nc.vector.tensor_tensor(out=c1[:], in0=sig_sb[:, 1024:2048], in1=c1[:], op=MUL)
                    nc.vector.tensor_tensor(out=c1[:], in0=c1[:], in1=t1[:], op=ADD)
                    tc_sb = kp.tile([128, 1024], F32, tag="tc_sb")
                    nc.scalar.activation(tc_sb[:], c1[:], TANH)
                    h_cur = hp.tile([128, 1024], F32, tag="h1o")
                    nc.gpsimd.tensor_tensor(out=h_cur[:], in0=sig_sb[:, 2048:3072], in1=tc_sb[:], op=MUL)
                    nc.sync.dma_start(out=h1_buf[0, s], in_=h_cur[:, 0:512])
                    nc.sync.dma_start(out=h1_buf[1, T - 1 - s], in_=h_cur[:, 512:1024])
                    h_prev = h_cur
                nc.vector.tensor_copy(out=hn1[:], in_=h_prev[:, 0:512])

            # ================= LSTM2 (one dir, B=512) =================
            with (
                tc.tile_pool(name="l2ring", bufs=8) as rp2,
                tc.tile_pool(name="l2work", bufs=3) as kp2,
                tc.tile_pool(name="l2state", bufs=1) as lsp2,
                tc.tile_pool(name="l2psum", bufs=2, space="PSUM") as pp2,
            ):
                c2 = lsp2.tile([128, 512], F32)
                nc.vector.memset(c2[:], 0.0)
                h2p = lsp2.tile([128, 512], F32)
                nc.vector.memset(h2p[:], 0.0)
                h2n = lsp2.tile([128, 512], F32)
                for s in range(T):
                    xf = rp2.tile([128, 512], F32, tag="xf")
                    nc.sync.dma_start(out=xf[:], in_=h1_buf[0, s])
                    xb = rp2.tile([128, 512], F32, tag="xb")
                    nc.sync.dma_start(out=xb[:], in_=h1_buf[1, s])
                    sp2t = pp2.tile([128, 1536], F32, tag="sp2", space="PSUM")
                    gp2 = pp2.tile([128, 512], F32, tag="gp2", space="PSUM")
                    for g, dst_info in ((0, (sp2t, 0)), (1, (sp2t, 512)), (3, (sp2t, 1024)), (2, (gp2, 0))):
                        dtile, off = dst_info
                        dst = dtile[:, off : off + 512]
                        nc.tensor.matmul(dst, lhsT=l2wih[:, 128 * g : 128 * g + 128], rhs=xf[:], start=True, stop=False)
                        nc.tensor.matmul(dst, lhsT=l2wih[:, 512 + 128 * g : 512 + 128 * g + 128], rhs=xb[:], start=False, stop=False)
                        nc.tensor.matmul(dst, lhsT=l2whh[:, 128 * g : 128 * g + 128], rhs=h2p[:], start=False, stop=True)
                    sb2 = kp2.tile([128, 1536], F32, tag="sb2")
                    nc.scalar.activation(sb2[:, 0:512], sp2t[:, 0:512], SIG, bias=l2b[:, 0:1])
                    nc.scalar.activation(sb2[:, 512:1024], sp2t[:, 512:1024], SIG, bias=l2b[:, 1:2])
                    nc.scalar.activation(sb2[:, 1024:1536], sp2t[:, 1024:1536], SIG, bias=l2b[:, 3:4])
                    g2sb = kp2.tile([128, 512], F32, tag="g2sb")
                    nc.scalar.activation(g2sb[:], gp2[:], TANH, bias=l2b[:, 2:3])
                    t2 = kp2.tile([128, 512], F32, tag="t2")
                    nc.vector.tensor_tensor(out=t2[:], in0=sb2[:, 0:512], in1=g2sb[:], op=MUL)
                    nc.vector.tensor_tensor(out=c2[:], in0=sb2[:, 512:1024], in1=c2[:], op=MUL)
                    nc.vector.tensor_tensor(out=c2[:], in0=c2[:], in1=t2[:], op=ADD)
                    tc2 = kp2.tile([128, 512], F32, tag="tc2")
                    nc.scalar.activation(tc2[:], c2[:], TANH)
                    dst_h = hn2cap if s == T - 1 else (h2n if s % 2 == 0 else h2p)
                    nc.gpsimd.tensor_tensor(out=dst_h[:], in0=sb2[:, 1024:1536], in1=tc2[:], op=MUL)
                    h2p, h2n = dst_h, (h2p if s % 2 == 0 else h2n)

            # ================= encoder tail =================
            with (
                tc.tile_pool(name="etwork", bufs=1) as ep,
                tc.tile_pool(name="etpsum", bufs=1, space="PSUM") as epp,
            ):
                hnsum = ep.tile([128, BE], F32)
                nc.vector.tensor_tensor(out=hnsum[:], in0=hn1[:], in1=hn2cap[:], op=ADD)
                X = ep.tile([128, 512], F32)
                hv = hnsum[:].rearrange("p (k two) -> p two k", two=2)
                nc.vector.tensor_copy(out=X[:, 0:256], in_=hv[:, 0, :])
                nc.vector.tensor_copy(out=X[:, 256:512], in_=hv[:, 1, :])
                fc1p = epp.tile([128, 512], F32, tag="fc1p", space="PSUM")
                for m in (0, 1):
                    dst = fc1p[:, 256 * m : 256 * m + 256]
                    nc.tensor.matmul(dst, lhsT=fc1w[:, 128 * m : 128 * m + 128], rhs=X[:, 0:256], start=True, stop=False)
                    nc.tensor.matmul(dst, lhsT=fc1w[:, 256 + 128 * m : 256 + 128 * m + 128], rhs=X[:, 256:512], start=False, stop=True)
                Y = ep.tile([128, 512], F32)
                nc.scalar.activation(Y[:, 0:256], fc1p[:, 0:256], SIG, bias=fc1b[:, 0:1])
                nc.scalar.activation(Y[:, 256:512], fc1p[:, 256:512], SIG, bias=fc1b[:, 1:2])
                fc2p = epp.tile([64, 256], F32, tag="fc2p", space="PSUM")
                nc.tensor.matmul(fc2p[:], lhsT=fc2w[:, 0:64], rhs=Y[:, 0:256], start=True, stop=False)
                nc.tensor.matmul(fc2p[:], lhsT=fc2w[:, 64:128], rhs=Y[:, 256:512], start=False, stop=True)
                nc.scalar.activation(hinit[0:64, :], fc2p[:], SIG, bias=fc2b[:])
                # node embeddings are gathered host-side; just load the slice
                nc.sync.dma_start(out=hinit[64:128, :], in_=node_embT_i[:])

            # ================= decoder (B=256) =================
            with (
                tc.tile_pool(name="dwork", bufs=3) as dp_pool,
                tc.tile_pool(name="dpsum", bufs=2, space="PSUM") as dpp,
                tc.tile_pool(name="dpsum1", bufs=1, space="PSUM") as dpp1,
            ):
                nc.vector.memset(h2g[:], 0.0)
                nc.sync.dma_start(out=res[:], in_=dec_init_i[:])
                ones256 = ones[:, 0:BD]
                for t in range(T):
                    g1p = dpp.tile([128, 1024], F32, tag="g1p", space="PSUM")
                    nc.tensor.matmul(g1p[:, 0:256], lhsT=g1whh[:, 0:128], rhs=hinit[:], start=True, stop=False)
                    nc.tensor.matmul(g1p[:, 0:256], lhsT=g1x[:, 0:128], rhs=res[:], start=False, stop=True)
                    nc.tensor.matmul(g1p[:, 256:512], lhsT=g1whh[:, 128:256], rhs=hinit[:], start=True, stop=False)
                    nc.tensor.matmul(g1p[:, 256:512], lhsT=g1x[:, 128:256], rhs=res[:], start=False, stop=True)
                    nc.tensor.matmul(g1p[:, 512:768], lhsT=g1x[:, 256:384], rhs=res[:], start=True, stop=True)
                    nc.tensor.matmul(g1p[:, 768:1024], lhsT=g1whh[:, 256:384], rhs=hinit[:], start=True, stop=False)
                    nc.tensor.matmul(g1p[:, 768:1024], lhsT=g1bhhn[:], rhs=ones256, start=False, stop=True)
                    rz_sb = dp_pool.tile([128, 512], F32, tag="rz_sb")
                    nc.scalar.activation(rz_sb[:], g1p[:, 0:512], SIG)
                    tt = dp_pool.tile([128, 256], F32, tag="tt")
                    nc.vector.tensor_tensor(out=tt[:], in0=rz_sb[:, 0:256], in1=g1p[:, 768:1024], op=MUL)
                    nc.vector.tensor_tensor(out=tt[:], in0=tt[:], in1=g1p[:, 512:768], op=ADD)
                    n_sb = dp_pool.tile([128, 256], F32, tag="n_sb")
                    nc.scalar.activation(n_sb[:], tt[:], TANH)
                    dtl = dp_pool.tile([128, 256], F32, tag="dtl")
                    nc.gpsimd.tensor_tensor(out=dtl[:], in0=hinit[:], in1=n_sb[:], op=SUB)
                    nc.gpsimd.tensor_tensor(out=dtl[:], in0=rz_sb[:, 256:512], in1=dtl[:], op=MUL)
                    nc.gpsimd.tensor_tensor(out=hinit[:], in0=n_sb[:], in1=dtl[:], op=ADD)
                    # GRU2
                    g2p = dpp1.tile([50, 1024], F32, tag="g2p", space="PSUM")
                    nc.tensor.matmul(g2p[:, 0:256], lhsT=g2wx[:, 0:50], rhs=hinit[:], start=True, stop=False)
                    nc.tensor.matmul(g2p[:, 0:256], lhsT=g2whh[:, 0:50], rhs=h2g[:], start=False, stop=False)
                    nc.tensor.matmul(g2p[:, 0:256], lhsT=g2brz[:, 0:50], rhs=ones256, start=False, stop=True)
                    nc.tensor.matmul(g2p[:, 256:512], lhsT=g2wx[:, 50:100], rhs=hinit[:], start=True, stop=False)
                    nc.tensor.matmul(g2p[:, 256:512], lhsT=g2whh[:, 50:100], rhs=h2g[:], start=False, stop=False)
                    nc.tensor.matmul(g2p[:, 256:512], lhsT=g2brz[:, 50:100], rhs=ones256, start=False, stop=True)
                    nc.tensor.matmul(g2p[:, 512:768], lhsT=g2wx[:, 100:150], rhs=hinit[:], start=True, stop=True)
                    nc.tensor.matmul(g2p[:, 768:1024], lhsT=g2whh[:, 100:150], rhs=h2g[:], start=True, stop=False)
                    nc.tensor.matmul(g2p[:, 768:1024], lhsT=g2bhhn[:], rhs=ones256, start=False, stop=True)
                    rz2 = dp_pool.tile([50, 512], F32, tag="rz2")
                    nc.scalar.activation(rz2[:], g2p[:, 0:512], SIG)
                    t2t = dp_pool.tile([50, 256], F32, tag="t2t")
                    nc.vector.tensor_tensor(out=t2t[:], in0=rz2[:, 0:256], in1=g2p[:, 768:1024], op=MUL)
                    nc.vector.tensor_tensor(out=t2t[:], in0=t2t[:], in1=g2p[:, 512:768], op=ADD)
                    n2 = dp_pool.tile([50, 256], F32, tag="n2")
                    nc.scalar.activation(n2[:], t2t[:], TANH, bias=g2bn[:])
                    d2 = dp_pool.tile([50, 256], F32, tag="d2")
                    nc.vector.tensor_tensor(out=d2[:], in0=h2g[:], in1=n2[:], op=SUB)
                    nc.vector.tensor_tensor(out=d2[:], in0=rz2[:, 256:512], in1=d2[:], op=MUL)
                    nc.vector.tensor_tensor(out=h2g[:], in0=n2[:], in1=d2[:], op=ADD)
                    # dec fc (replicated rows)
                    dcp = dpp.tile([128, 256], F32, tag="dcp", space="PSUM")
                    nc.tensor.matmul(dcp[:], lhsT=decw[:], rhs=h2g[:], start=True, stop=True)
                    nc.scalar.activation(res[0:1, :], dcp[0:1, :], SIG, bias=decb[0:1, :])
                    k = t % 8
                    if k == 0:
                        oblk = dp_pool.tile([1, 8 * BD], F16, tag="oblk")
                    nc.scalar.activation(
                        oblk[0:1, BD * k : BD * k + BD], dcp[0:1, :], SIG,
                        bias=decb[0:1, :])
                    if k == 7:
                        nc.sync.dma_start(out=out_staged[t // 8], in_=oblk[:])

    nc.finalize()
    return nc


def _prep_static(inputs):
    """Per-core weight-derived arrays (everything except DYNAMIC tensors)."""
    inp = {k: np.asarray(v) for k, v in inputs.items()}

    def lstm1_dir(d):  # d in 'fb'
        whhT = np.ascontiguousarray(inp[f"l1_whh_{d}"].T.astype(np.float32))  # (128, 512)
        wx = inp[f"l1_wih_{d}"][:, 0].astype(np.float32)  # (512,)
        bias = inp[f"l1_b_{d}"].astype(np.float32).reshape(4, 128).T  # (128, 4)
        return whhT, wx, bias

    wf, wxf, bf = lstm1_dir("f")
    wb, wxb, bb = lstm1_dir("b")

    g1_bias = np.concatenate(
        [(inp["g1_bih"] + inp["g1_bhh"])[0:256], inp["g1_bih"][256:384]])
    shared = dict(
        fc1_wT=np.ascontiguousarray(inp["fc1_w"].T.reshape(2, 128, 256)),
        fc1_bias=np.ascontiguousarray(inp["fc1_b"].reshape(2, 128).T),
        fc2_wT=np.ascontiguousarray(inp["fc2_w"].T.reshape(2, 128, 64)),
        fc2_bias=inp["fc2_b"][:, None].astype(np.float32),
        g1_whhT=np.ascontiguousarray(inp["g1_whh"].T),
        g1_xaug=np.ascontiguousarray(np.stack([inp["g1_wih"][:, 0], g1_bias])),
        g1_bhhn=np.ascontiguousarray(inp["g1_bhh"][None, 256:384]),
        g2_wxT=np.ascontiguousarray(inp["g2_wih"].T),
        g2_whhT=np.ascontiguousarray(inp["g2_whh"].T),
        g2_brz=np.ascontiguousarray((inp["g2_bih"] + inp["g2_bhh"])[None, 0:100]),
        g2_bn=np.ascontiguousarray(inp["g2_bih"][100:150, None]),
        g2_bhhn=np.ascontiguousarray(inp["g2_bhh"][None, 100:150]),
        dec_wTr=np.ascontiguousarray(np.repeat(inp["dec_w"].T, 128, axis=1)),
        dec_br=np.ascontiguousarray(np.repeat(inp["dec_b"][:, None], 128, axis=0)),
    )
    maps = []
    for c in range(NC):
        rev = c >= 4
        d2 = "b" if rev else "f"  # LSTM2 direction this core needs
        m = dict(shared)
        m["l1_whhT"] = np.ascontiguousarray(np.stack([wb, wf] if rev else [wf, wb]))
        m["l1_wxT"] = np.ascontiguousarray(
            np.concatenate([wxb, wxf] if rev else [wxf, wxb])[None, :])
        m["l1_bias"] = np.ascontiguousarray(
            np.concatenate([bb, bf] if rev else [bf, bb], axis=1))
        m["l2_wihT"] = np.ascontiguousarray(
            inp[f"l2_wih_{d2}"].T.reshape(2, 128, 512).astype(np.float32))
        m["l2_whhT"] = np.ascontiguousarray(inp[f"l2_whh_{d2}"].T)
        m["l2_bias"] = np.ascontiguousarray(inp[f"l2_b_{d2}"].reshape(4, 128).T)
        maps.append(m)
    return maps


def _prep_dynamic(inputs):
    """Per-core arrays that change every call: edge seqs, gathered node embs."""
    edge = np.asarray(inputs["edge_data"])[:, :, 0].astype(np.float32)  # (2048, 200)
    node = np.asarray(inputs["node_data"]).astype(np.int64)
    emb = np.asarray(inputs["emb"]).astype(np.float32)
    node_emb = 0.5 * (emb[node[:, 0]] + emb[node[:, 1]])  # (2048, 64)
    maps = []
    for c in range(NC):
        cp = c % 4
        J = slice(512 * cp, 512 * cp + 512)
        I = slice(256 * c, 256 * c + 256)
        eT = edge[J].T  # (200, 512)
        if c >= 4:
            eT = eT[::-1]
        maps.append(dict(
            edge_src=np.ascontiguousarray(eT.astype(np.float16)),
            node_embT=np.ascontiguousarray(node_emb[I].T),
            dec_init=np.ascontiguousarray(
                np.stack([edge[I, -1], np.ones(BD, np.float32)])),
        ))
    return maps


def _weights_key(inputs):
    h = hashlib.sha1()
    for k in sorted(inputs):
        if k in ("node_data", "edge_data", "emb"):
            continue
        a = np.asarray(inputs[k])
        h.update(k.encode())
        h.update(a.tobytes())
    return h.hexdigest()


def _get_state():
    if "state" in _CACHE:
        return _CACHE["state"]
    nc = _build_program()
    b2j.install_neuronx_cc_hook()

    partition_name = nc.partition_id_tensor.name if nc.partition_id_tensor else None
    in_names, out_names, out_avals = [], [], []
    for alloc in nc.m.functions[0].allocations:
        if not isinstance(alloc, mybir.MemoryLocationSet):
            continue
        name = alloc.memorylocations[0].name
        if alloc.kind == "ExternalInput":
            if name != partition_name:
                in_names.append(name)
        elif alloc.kind == "ExternalOutput":
            out_names.append(name)
            out_avals.append(jax.core.ShapedArray(
                tuple(alloc.tensor_shape), mybir.dt.np(alloc.dtype)))
    in_names_all = list(in_names) + ([partition_name] if partition_name else [])

    def _body(*args):
        operands = list(args)
        if partition_name is not None:
            operands.append(b2j.partition_id_tensor())
        outs = b2j._bass_exec_p.bind(
            *operands, out_avals=tuple(out_avals), in_names=tuple(in_names_all),
            out_names=tuple(out_names), lowering_input_output_aliases=(),
            sim_require_finite=True, sim_require_nnan=True, nc=nc)
        return tuple(outs)

    devices = jax.devices()[:NC]
    mesh = Mesh(np.asarray(devices), ("core",))
    jitted = jax.jit(
        jax.shard_map(_body, mesh=mesh,
                      in_specs=(PartitionSpec("core"),) * len(in_names),
                      out_specs=(PartitionSpec("core"),) * len(out_names),
                      check_vma=False),
        keep_unused=True)

    # shapes/dtypes of the global (concat over cores) inputs, for lowering
    name_to_alloc = {}
    for alloc in nc.m.functions[0].allocations:
        if isinstance(alloc, mybir.MemoryLocationSet) and alloc.kind == "ExternalInput":
            name_to_alloc[alloc.memorylocations[0].name] = alloc
    sds = [
        jax.ShapeDtypeStruct(
            (NC * name_to_alloc[n].tensor_shape[0], *name_to_alloc[n].tensor_shape[1:]),
            mybir.dt.np(name_to_alloc[n].dtype))
        for n in in_names
    ]
    compiled = jitted.lower(*sds).compile()

    state = dict(
        compiled=compiled, in_names=in_names, out_names=out_names,
        sharding=NamedSharding(mesh, PartitionSpec("core")),
        static_key=None, static_dev={},
    )
    _CACHE["state"] = state
    return state


def _concat(maps, name):
    return np.concatenate([np.asarray(m[name]) for m in maps], axis=0)


def run_device(inputs, trace=False):
    st = _get_state()
    key = _weights_key(inputs)
    if st["static_key"] != key:
        smaps = _prep_static(inputs)
        dev = {}
        for n in smaps[0]:
            dev[n] = jax.device_put(_concat(smaps, n), st["sharding"])
        jax.block_until_ready(list(dev.values()))
        st["static_dev"] = dev
        st["static_key"] = key
    dmaps = _prep_dynamic(inputs)
    args = []
    for n in st["in_names"]:
        if n in DYNAMIC:
            args.append(_concat(dmaps, n))
        else:
            args.append(st["static_dev"][n])
    out_arrs = st["compiled"](*args)
    staged = np.asarray(out_arrs[0]).reshape(NC, 25, 8, BD)  # (core, blk, k, b)
    out = np.zeros((2048, T, 1), np.float32)
    for c in range(NC):
        out[256 * c : 256 * c + 256, :, 0] = staged[c].reshape(T, BD).T
    return out, None


def kernel(**inputs) -> np.ndarray:
    out, _ = run_device(inputs)
    return out


# revision 10
# speedup vs baseline: 29.7104x; 1.0493x over previous
# Trainium2 Bass kernel for nn_Graph_AutoEncoder (BiLSTM encoder + GRU decoder).
#
# Sharding: decoder rows i in [256c, 256c+256) per core c. Each core encodes the
# 512 batch rows j = 2i, 2i+1 its decoder slice needs (LSTM1 both dirs at B=512,
# LSTM2 one direction). Cores 4-7 need the *backward* LSTM2 direction; they get
# time-reversed edge sequences and f/b-swapped LSTM1 weights via their input map,
# so the compiled program is identical on all 8 cores (SPMD, no collectives).
#
# Host/transfer strategy (the dominant cost on axon-tunneled cores): the PJRT
# executable is compiled once and cached; all weight-derived tensors are kept
# device-resident across calls (re-uploaded only when the weight bytes change);
# the 50k x 64 embedding lookup happens on host so only the gathered (64, 256)
# slice per core is shipped. Per call only edge data + gathered node embeddings
# + decoder init (~2 MB total) cross the tunnel.
#
# Layout: feature-on-partition. Gates are computed as W @ h matmuls into PSUM
# (lhsT = W^T with K on partitions); biases are folded into ScalarE activation
# bias operands (LSTM1/LSTM2) or bias-row matmuls against an on-chip ones tile
# (decoder GRUs). Matmul operands (weights, hidden states, edge inputs) are
# fp16 for the 1-cycle/row PE mode (fp32 runs at 1/4 rate); PSUM accumulation,
# cell states, biases and all elementwise math stay fp32.
import hashlib

import numpy as np

import jax
from jax.sharding import Mesh, NamedSharding, PartitionSpec

import concourse.bass2jax as b2j
import concourse.mybir as mybir
import concourse.tile as tile
from concourse import bacc

F32 = mybir.dt.float32
F16 = mybir.dt.float16
SIG = mybir.ActivationFunctionType.Sigmoid
TANH = mybir.ActivationFunctionType.Tanh
MUL = mybir.AluOpType.mult
ADD = mybir.AluOpType.add
SUB = mybir.AluOpType.subtract

T = 200
NC = 8
BE = 512  # encoder batch per core
BD = 256  # decoder batch per core

# Inputs that change per call; everything else is weight-derived and cached
# on-device between calls.
DYNAMIC = ("edge_src", "node_embT", "dec_init")

_CACHE = {}


def _build_program():
    nc = bacc.Bacc("TRN2", target_bir_lowering=False, debug=False, num_devices=NC)

    def din(name, shape, d=F16):
        return nc.dram_tensor(name, shape, d, kind="ExternalInput").ap()

    edge_src = din("edge_src", [T, BE])
    node_embT_i = din("node_embT", [64, BD])
    dec_init_i = din("dec_init", [2, BD])
    l1_whhT = din("l1_whhT", [2, 128, 512])
    l1_wxT_i = din("l1_wxT", [1, 1024])
    l1_bias_i = din("l1_bias", [128, 8], F32)
    l2_wihT = din("l2_wihT", [2, 128, 512])
    l2_whhT_i = din("l2_whhT", [128, 512])
    l2_bias_i = din("l2_bias", [128, 4], F32)
    fc1_wT = din("fc1_wT", [2, 128, 256])
    fc1_bias_i = din("fc1_bias", [128, 2], F32)
    fc2_wT = din("fc2_wT", [2, 128, 64])
    fc2_bias_i = din("fc2_bias", [64, 1], F32)
    g1_whhT_i = din("g1_whhT", [128, 384])
    g1_xaug_i = din("g1_xaug", [2, 384])
    g1_bhhn_i = din("g1_bhhn", [1, 128])
    g2_wxT_i = din("g2_wxT", [128, 150])
    g2_whhT_i = din("g2_whhT", [50, 150])
    g2_brz_i = din("g2_brz", [1, 100])
    g2_bn_i = din("g2_bn", [50, 1], F32)
    g2_bhhn_i = din("g2_bhhn", [1, 50])
    dec_wTr_i = din("dec_wTr", [50, 128])
    dec_br_i = din("dec_br", [128, 1], F32)

    out_staged = nc.dram_tensor("out_staged", [25, 8 * BD], F16, kind="ExternalOutput").ap()
    h1_buf = nc.dram_tensor("h1_buf", [2, T, 128, BE], F16).ap()

    with tile.TileContext(nc) as tc:
        with (
            tc.tile_pool(name="wpool", bufs=1) as wp,
            tc.tile_pool(name="spool", bufs=1) as sp,
        ):
            # ---- persistent weights (fp16 matmul operands, fp32 biases) ----
            l1w = wp.tile([128, 1024], F16)
            nc.sync.dma_start(out=l1w[:, 0:512], in_=l1_whhT[0])
            nc.sync.dma_start(out=l1w[:, 512:1024], in_=l1_whhT[1])
            l1wx = wp.tile([1, 1024], F16)
            nc.sync.dma_start(out=l1wx[:], in_=l1_wxT_i[:])
            l1b = wp.tile([128, 8], F32)
            nc.sync.dma_start(out=l1b[:], in_=l1_bias_i[:])
            l2wih = wp.tile([128, 1024], F16)
            nc.sync.dma_start(out=l2wih[:, 0:512], in_=l2_wihT[0])
            nc.sync.dma_start(out=l2wih[:, 512:1024], in_=l2_wihT[1])
            l2whh = wp.tile([128, 512], F16)
            nc.sync.dma_start(out=l2whh[:], in_=l2_whhT_i[:])
            l2b = wp.tile([128, 4], F32)
            nc.sync.dma_start(out=l2b[:], in_=l2_bias_i[:])
            fc1w = wp.tile([128, 512], F16)
            nc.sync.dma_start(out=fc1w[:, 0:256], in_=fc1_wT[0])
            nc.sync.dma_start(out=fc1w[:, 256:512], in_=fc1_wT[1])
            fc1b = wp.tile([128, 2], F32)
            nc.sync.dma_start(out=fc1b[:], in_=fc1_bias_i[:])
            fc2w = wp.tile([128, 128], F16)
            nc.sync.dma_start(out=fc2w[:, 0:64], in_=fc2_wT[0])
            nc.sync.dma_start(out=fc2w[:, 64:128], in_=fc2_wT[1])
            fc2b = wp.tile([64, 1], F32)
            nc.sync.dma_start(out=fc2b[:], in_=fc2_bias_i[:])
            g1whh = wp.tile([128, 384], F16)
            nc.sync.dma_start(out=g1whh[:], in_=g1_whhT_i[:])
            g1x = wp.tile([2, 384], F16)
            nc.sync.dma_start(out=g1x[:], in_=g1_xaug_i[:])
            g1bhhn = wp.tile([1, 128], F16)
            nc.sync.dma_start(out=g1bhhn[:], in_=g1_bhhn_i[:])
            g2wx = wp.tile([128, 150], F16)
            nc.sync.dma_start(out=g2wx[:], in_=g2_wxT_i[:])
            g2whh = wp.tile([50, 150], F16)
            nc.sync.dma_start(out=g2whh[:], in_=g2_whhT_i[:])
            g2brz = wp.tile([1, 100], F16)
            nc.sync.dma_start(out=g2brz[:], in_=g2_brz_i[:])
            g2bn = wp.tile([50, 1], F32)
            nc.sync.dma_start(out=g2bn[:], in_=g2_bn_i[:])
            g2bhhn = wp.tile([1, 50], F16)
            nc.sync.dma_start(out=g2bhhn[:], in_=g2_bhhn_i[:])
            decw = wp.tile([50, 128], F16)
            nc.sync.dma_start(out=decw[:], in_=dec_wTr_i[:])
            decb = wp.tile([128, 1], F32)
            nc.sync.dma_start(out=decb[:], in_=dec_br_i[:])
            ones = wp.tile([1, BE], F16)
            nc.vector.memset(ones[:], 1.0)

            # ---- persistent state ----
            hn1 = sp.tile([128, BE], F32)
            hn2cap = sp.tile([128, BE], F32)
            hinit = sp.tile([128, BD], F16)
            h2g = sp.tile([50, BD], F16)
            res = sp.tile([2, BD], F16)

            # ================= LSTM1 (both dirs, B=512) =================
            with (
                tc.tile_pool(name="l1ring", bufs=8) as rp,
                tc.tile_pool(name="l1hring", bufs=4) as hp,
                tc.tile_pool(name="l1work", bufs=3) as kp,
                tc.tile_pool(name="l1state", bufs=1) as lsp,
                tc.tile_pool(name="l1psum", bufs=1, space="PSUM") as pp,
            ):
                c1 = lsp.tile([128, 1024], F32)
                nc.vector.memset(c1[:], 0.0)
                h_prev = hp.tile([128, 1024], F16, tag="h1o")
                nc.vector.memset(h_prev[:], 0.0)
                for s in range(T):
                    erf = rp.tile([1, BE], F16, tag="erf")
                    nc.sync.dma_start(out=erf[:], in_=edge_src[s : s + 1])
                    erb = rp.tile([1, BE], F16, tag="erb")
                    nc.sync.dma_start(out=erb[:], in_=edge_src[T - 1 - s : T - s])
                    sigp = pp.tile([128, 3072], F32, tag="sigp", space="PSUM")
                    gp = pp.tile([128, 1024], F32, tag="gp", space="PSUM")
                    for d in (0, 1):
                        hs_d = h_prev[:, 512 * d : 512 * d + 512]
                        er_d = erf if d == 0 else erb
                        for gi, g in ((0, 0), (1, 1), (2, 3), (3, 2)):
                            if g == 2:  # tanh gate
                                dst = gp[:, 512 * d : 512 * d + 512]
                            else:
                                col = (0, 1, None, 2)[g]
                                dst = sigp[:, 1024 * col + 512 * d : 1024 * col + 512 * d + 512]
                            nc.tensor.matmul(
                                dst, lhsT=l1w[:, 512 * d + 128 * g : 512 * d + 128 * g + 128],
                                rhs=hs_d, start=True, stop=False)
                            nc.tensor.matmul(
                                dst, lhsT=l1wx[:, 512 * d + 128 * g : 512 * d + 128 * g + 128],
                                rhs=er_d[:], start=False, stop=True)
                    sig_sb = kp.tile([128, 3072], F32, tag="sig_sb")
                    for d in (0, 1):
                        for g in (0, 1, 3):
                            col = (0, 1, None, 2)[g]
                            off = 1024 * col + 512 * d
                            nc.scalar.activation(
                                sig_sb[:, off : off + 512], sigp[:, off : off + 512],
                                SIG, bias=l1b[:, 4 * d + g : 4 * d + g + 1])
                    g_sb = kp.tile([128, 1024], F32, tag="g_sb")
                    for d in (0, 1):
                        nc.scalar.activation(
                            g_sb[:, 512 * d : 512 * d + 512], gp[:, 512 * d : 512 * d + 512],
                            TANH, bias=l1b[:, 4 * d + 2 : 4 * d + 3])
                    t1 = kp.tile([128, 1024], F32, tag="t1")
                    nc.vector.tensor_tensor(out=t1[:], in0=sig_sb[:, 0:1024], in1=g_sb[:], op=MUL)
                    nc.vector.tensor_tensor(out=c1[:], in0=sig_sb[:, 1024:2048], in1=c1[:], op=MUL)
                    nc.vector.tensor_tensor(out=c1[:], in0=c1[:], in1=t1[:], op=ADD)
                    tc_sb = kp.tile([128, 1024], F32, tag="tc_sb")
                    nc.scalar.activation(tc_sb[:], c1[:], TANH)
                    h_cur = hp.tile([128, 1024], F16, tag="h1o")
                    nc.gpsimd.tensor_tensor(out=h_cur[:], in0=sig_sb[:, 2048:3072], in1=tc_sb[:], op=MUL)
                    nc.sync.dma_start(out=h1_buf[0, s], in_=h_cur[:, 0:512])
                    nc.sync.dma_start(out=h1_buf[1, T - 1 - s], in_=h_cur[:, 512:1024])
                    h_prev = h_cur
                nc.vector.tensor_copy(out=hn1[:], in_=h_prev[:, 0:512])

            # ================= LSTM2 (one dir, B=512) =================
            with (
                tc.tile_pool(name="l2ring", bufs=8) as rp2,
                tc.tile_pool(name="l2work", bufs=3) as kp2,
                tc.tile_pool(name="l2state", bufs=1) as lsp2,
                tc.tile_pool(name="l2psum", bufs=2, space="PSUM") as pp2,
            ):
                c2 = lsp2.tile([128, 512], F32)
                nc.vector.memset(c2[:], 0.0)
                h2p = lsp2.tile([128, 512], F16)
                nc.vector.memset(h2p[:], 0.0)
                h2n = lsp2.tile([128, 512], F16)
                hn2c16 = lsp2.tile([128, 512], F16)
                for s in range(T):
                    xf = rp2.tile([128, 512], F16, tag="xf")
                    nc.sync.dma_start(out=xf[:], in_=h1_buf[0, s])
                    xb = rp2.tile([128, 512], F16, tag="xb")
                    nc.sync.dma_start(out=xb[:], in_=h1_buf[1, s])
                    sp2t = pp2.tile([128, 1536], F32, tag="sp2", space="PSUM")
                    gp2 = pp2.tile([128, 512], F32, tag="gp2", space="PSUM")
                    for g, dst_info in ((0, (sp2t, 0)), (1, (sp2t, 512)), (3, (sp2t, 1024)), (2, (gp2, 0))):
                        dtile, off = dst_info
                        dst = dtile[:, off : off + 512]
                        nc.tensor.matmul(dst, lhsT=l2wih[:, 128 * g : 128 * g + 128], rhs=xf[:], start=True, stop=False)
                        nc.tensor.matmul(dst, lhsT=l2wih[:, 512 + 128 * g : 512 + 128 * g + 128], rhs=xb[:], start=False, stop=False)
                        nc.tensor.matmul(dst, lhsT=l2whh[:, 128 * g : 128 * g + 128], rhs=h2p[:], start=False, stop=True)
                    sb2 = kp2.tile([128, 1536], F32, tag="sb2")
                    nc.scalar.activation(sb2[:, 0:512], sp2t[:, 0:512], SIG, bias=l2b[:, 0:1])
                    nc.scalar.activation(sb2[:, 512:1024], sp2t[:, 512:1024], SIG, bias=l2b[:, 1:2])
                    nc.scalar.activation(sb2[:, 1024:1536], sp2t[:, 1024:1536], SIG, bias=l2b[:, 3:4])
                    g2sb = kp2.tile([128, 512], F32, tag="g2sb")
                    nc.scalar.activation(g2sb[:], gp2[:], TANH, bias=l2b[:, 2:3])
                    t2 = kp2.tile([128, 512], F32, tag="t2")
                    nc.vector.tensor_tensor(out=t2[:], in0=sb2[:, 0:512], in1=g2sb[:], op=MUL)
                    nc.vector.tensor_tensor(out=c2[:], in0=sb2[:, 512:1024], in1=c2[:], op=MUL)
                    nc.vector.tensor_tensor(out=c2[:], in0=c2[:], in1=t2[:], op=ADD)
                    tc2 = kp2.tile([128, 512], F32, tag="tc2")
                    nc.scalar.activation(tc2[:], c2[:], TANH)
                    dst_h = hn2c16 if s == T - 1 else (h2n if s % 2 == 0 else h2p)
                    nc.gpsimd.tensor_tensor(out=dst_h[:], in0=sb2[:, 1024:1536], in1=tc2[:], op=MUL)
                    h2p, h2n = dst_h, (h2p if s % 2 == 0 else h2n)
                nc.vector.tensor_copy(out=hn2cap[:], in_=hn2c16[:])

            # ================= encoder tail =================
            with (
                tc.tile_pool(name="etwork", bufs=1) as ep,
                tc.tile_pool(name="etpsum", bufs=1, space="PSUM") as epp,
            ):
                hnsum = ep.tile([128, BE], F32)
                nc.vector.tensor_tensor(out=hnsum[:], in0=hn1[:], in1=hn2cap[:], op=ADD)
                X = ep.tile([128, 512], F16)
                hv = hnsum[:].rearrange("p (k two) -> p two k", two=2)
                nc.vector.tensor_copy(out=X[:, 0:256], in_=hv[:, 0, :])
                nc.vector.tensor_copy(out=X[:, 256:512], in_=hv[:, 1, :])
                fc1p = epp.tile([128, 512], F32, tag="fc1p", space="PSUM")
                for m in (0, 1):
                    dst = fc1p[:, 256 * m : 256 * m + 256]
                    nc.tensor.matmul(dst, lhsT=fc1w[:, 128 * m : 128 * m + 128], rhs=X[:, 0:256], start=True, stop=False)
                    nc.tensor.matmul(dst, lhsT=fc1w[:, 256 + 128 * m : 256 + 128 * m + 128], rhs=X[:, 256:512], start=False, stop=True)
                Y = ep.tile([128, 512], F16)
                nc.scalar.activation(Y[:, 0:256], fc1p[:, 0:256], SIG, bias=fc1b[:, 0:1])
                nc.scalar.activation(Y[:, 256:512], fc1p[:, 256:512], SIG, bias=fc1b[:, 1:2])
                fc2p = epp.tile([64, 256], F32, tag="fc2p", space="PSUM")
                nc.tensor.matmul(fc2p[:], lhsT=fc2w[:, 0:64], rhs=Y[:, 0:256], start=True, stop=False)
                nc.tensor.matmul(fc2p[:], lhsT=fc2w[:, 64:128], rhs=Y[:, 256:512], start=False, stop=True)
                nc.scalar.activation(hinit[0:64, :], fc2p[:], SIG, bias=fc2b[:])
                # node embeddings are gathered host-side; just load the slice
                nc.sync.dma_start(out=hinit[64:128, :], in_=node_embT_i[:])

            # ================= decoder (B=256) =================
            with (
                tc.tile_pool(name="dwork", bufs=3) as dp_pool,
                tc.tile_pool(name="dpsum", bufs=2, space="PSUM") as dpp,
                tc.tile_pool(name="dpsum1", bufs=1, space="PSUM") as dpp1,
            ):
                nc.vector.memset(h2g[:], 0.0)
                nc.sync.dma_start(out=res[:], in_=dec_init_i[:])
                ones256 = ones[:, 0:BD]
                for t in range(T):
                    g1p = dpp.tile([128, 1024], F32, tag="g1p", space="PSUM")
                    nc.tensor.matmul(g1p[:, 0:256], lhsT=g1whh[:, 0:128], rhs=hinit[:], start=True, stop=False)
                    nc.tensor.matmul(g1p[:, 0:256], lhsT=g1x[:, 0:128], rhs=res[:], start=False, stop=True)
                    nc.tensor.matmul(g1p[:, 256:512], lhsT=g1whh[:, 128:256], rhs=hinit[:], start=True, stop=False)
                    nc.tensor.matmul(g1p[:, 256:512], lhsT=g1x[:, 128:256], rhs=res[:], start=False, stop=True)
                    nc.tensor.matmul(g1p[:, 512:768], lhsT=g1x[:, 256:384], rhs=res[:], start=True, stop=True)
                    nc.tensor.matmul(g1p[:, 768:1024], lhsT=g1whh[:, 256:384], rhs=hinit[:], start=True, stop=False)
                    nc.tensor.matmul(g1p[:, 768:1024], lhsT=g1bhhn[:], rhs=ones256, start=False, stop=True)
                    rz_sb = dp_pool.tile([128, 512], F32, tag="rz_sb")
                    nc.scalar.activation(rz_sb[:], g1p[:, 0:512], SIG)
                    tt = dp_pool.tile([128, 256], F32, tag="tt")
                    nc.vector.tensor_tensor(out=tt[:], in0=rz_sb[:, 0:256], in1=g1p[:, 768:1024], op=MUL)
                    nc.vector.tensor_tensor(out=tt[:], in0=tt[:], in1=g1p[:, 512:768], op=ADD)
                    n_sb = dp_pool.tile([128, 256], F32, tag="n_sb")
                    nc.scalar.activation(n_sb[:], tt[:], TANH)
                    dtl = dp_pool.tile([128, 256], F32, tag="dtl")
                    nc.gpsimd.tensor_tensor(out=dtl[:], in0=hinit[:], in1=n_sb[:], op=SUB)
                    nc.gpsimd.tensor_tensor(out=dtl[:], in0=rz_sb[:, 256:512], in1=dtl[:], op=MUL)
                    nc.gpsimd.tensor_tensor(out=hinit[:], in0=n_sb[:], in1=dtl[:], op=ADD)
                    # GRU2
                    g2p = dpp1.tile([50, 1024], F32, tag="g2p", space="PSUM")
                    nc.tensor.matmul(g2p[:, 0:256], lhsT=g2wx[:, 0:50], rhs=hinit[:], start=True, stop=False)
                    nc.tensor.matmul(g2p[:, 0:256], lhsT=g2whh[:, 0:50], rhs=h2g[:], start=False, stop=False)
                    nc.tensor.matmul(g2p[:, 0:256], lhsT=g2brz[:, 0:50], rhs=ones256, start=False, stop=True)
                    nc.tensor.matmul(g2p[:, 256:512], lhsT=g2wx[:, 50:100], rhs=hinit[:], start=True, stop=False)
                    nc.tensor.matmul(g2p[:, 256:512], lhsT=g2whh[:, 50:100], rhs=h2g[:], start=False, stop=False)
                    nc.tensor.matmul(g2p[:, 256:512], lhsT=g2brz[:, 50:100], rhs=ones256, start=False, stop=True)
                    nc.tensor.matmul(g2p[:, 512:768], lhsT=g2wx[:, 100:150], rhs=hinit[:], start=True, stop=True)
                    nc.tensor.matmul(g2p[:, 768:1024], lhsT=g2whh[:, 100:150], rhs=h2g[:], start=True, stop=False)
                    nc.tensor.matmul(g2p[:, 768:1024], lhsT=g2bhhn[:], rhs=ones256, start=False, stop=True)
                    rz2 = dp_pool.tile([50, 512], F32, tag="rz2")
                    nc.scalar.activation(rz2[:], g2p[:, 0:512], SIG)
                    t2t = dp_pool.tile([50, 256], F32, tag="t2t")
                    nc.vector.tensor_tensor(out=t2t[:], in0=rz2[:, 0:256], in1=g2p[:, 768:1024], op=MUL)
                    nc.vector.tensor_tensor(out=t2t[:], in0=t2t[:], in1=g2p[:, 512:768], op=ADD)
                    n2 = dp_pool.tile([50, 256], F32, tag="n2")
                    nc.scalar.activation(n2[:], t2t[:], TANH, bias=g2bn[:])
                    d2 = dp_pool.tile([50, 256], F32, tag="d2")
                    nc.vector.tensor_tensor(out=d2[:], in0=h2g[:], in1=n2[:], op=SUB)
                    nc.vector.tensor_tensor(out=d2[:], in0=rz2[:, 256:512], in1=d2[:], op=MUL)
                    nc.vector.tensor_tensor(out=h2g[:], in0=n2[:], in1=d2[:], op=ADD)
                    # dec fc (replicated rows)
                    dcp = dpp.tile([128, 256], F32, tag="dcp", space="PSUM")
                    nc.tensor.matmul(dcp[:], lhsT=decw[:], rhs=h2g[:], start=True, stop=True)
                    nc.scalar.activation(res[0:1, :], dcp[0:1, :], SIG, bias=decb[0:1, :])
                    k = t % 8
                    if k == 0:
                        oblk = dp_pool.tile([1, 8 * BD], F16, tag="oblk")
                    nc.scalar.activation(
                        oblk[0:1, BD * k : BD * k + BD], dcp[0:1, :], SIG,
                        bias=decb[0:1, :])
                    if k == 7:
                        nc.sync.dma_start(out=out_staged[t // 8], in_=oblk[:])

    nc.finalize()
    return nc


def _prep_static(inputs):
    """Per-core weight-derived arrays (everything except DYNAMIC tensors)."""
    inp = {k: np.asarray(v) for k, v in inputs.items()}
    f16 = np.float16

    def lstm1_dir(d):  # d in 'fb'
        whhT = np.ascontiguousarray(inp[f"l1_whh_{d}"].T.astype(f16))  # (128, 512)
        wx = inp[f"l1_wih_{d}"][:, 0].astype(f16)  # (512,)
        bias = inp[f"l1_b_{d}"].astype(np.float32).reshape(4, 128).T  # (128, 4)
        return whhT, wx, bias

    wf, wxf, bf = lstm1_dir("f")
    wb, wxb, bb = lstm1_dir("b")

    g1_bias = np.concatenate(
        [(inp["g1_bih"] + inp["g1_bhh"])[0:256], inp["g1_bih"][256:384]])
    shared = dict(
        fc1_wT=np.ascontiguousarray(inp["fc1_w"].T.reshape(2, 128, 256).astype(f16)),
        fc1_bias=np.ascontiguousarray(inp["fc1_b"].reshape(2, 128).T.astype(np.float32)),
        fc2_wT=np.ascontiguousarray(inp["fc2_w"].T.reshape(2, 128, 64).astype(f16)),
        fc2_bias=inp["fc2_b"][:, None].astype(np.float32),
        g1_whhT=np.ascontiguousarray(inp["g1_whh"].T.astype(f16)),
        g1_xaug=np.ascontiguousarray(np.stack([inp["g1_wih"][:, 0], g1_bias]).astype(f16)),
        g1_bhhn=np.ascontiguousarray(inp["g1_bhh"][None, 256:384].astype(f16)),
        g2_wxT=np.ascontiguousarray(inp["g2_wih"].T.astype(f16)),
        g2_whhT=np.ascontiguousarray(inp["g2_whh"].T.astype(f16)),
        g2_brz=np.ascontiguousarray((inp["g2_bih"] + inp["g2_bhh"])[None, 0:100].astype(f16)),
        g2_bn=np.ascontiguousarray(inp["g2_bih"][100:150, None].astype(np.float32)),
        g2_bhhn=np.ascontiguousarray(inp["g2_bhh"][None, 100:150].astype(f16)),
        dec_wTr=np.ascontiguousarray(np.repeat(inp["dec_w"].T, 128, axis=1).astype(f16)),
        dec_br=np.ascontiguousarray(np.repeat(inp["dec_b"][:, None], 128, axis=0).astype(np.float32)),
    )
    maps = []
    for c in range(NC):
        rev = c >= 4
        d2 = "b" if rev else "f"  # LSTM2 direction this core needs
        m = dict(shared)
        m["l1_whhT"] = np.ascontiguousarray(np.stack([wb, wf] if rev else [wf, wb]))
        m["l1_wxT"] = np.ascontiguousarray(
            np.concatenate([wxb, wxf] if rev else [wxf, wxb])[None, :])
        m["l1_bias"] = np.ascontiguousarray(
            np.concatenate([bb, bf] if rev else [bf, bb], axis=1))
        m["l2_wihT"] = np.ascontiguousarray(
            inp[f"l2_wih_{d2}"].T.reshape(2, 128, 512).astype(f16))
        m["l2_whhT"] = np.ascontiguousarray(inp[f"l2_whh_{d2}"].T.astype(f16))
        m["l2_bias"] = np.ascontiguousarray(
            inp[f"l2_b_{d2}"].reshape(4, 128).T.astype(np.float32))
        maps.append(m)
    return maps


def _prep_dynamic(inputs):
    """Per-core arrays that change every call: edge seqs, gathered node embs."""
    edge = np.asarray(inputs["edge_data"])[:, :, 0].astype(np.float32)  # (2048, 200)
    node = np.asarray(inputs["node_data"]).astype(np.int64)
    emb = np.asarray(inputs["emb"]).astype(np.float32)
    node_emb = (0.5 * (emb[node[:, 0]] + emb[node[:, 1]])).astype(np.float16)
    maps = []
    for c in range(NC):
        cp = c % 4
        J = slice(512 * cp, 512 * cp + 512)
        I = slice(256 * c, 256 * c + 256)
        eT = edge[J].T  # (200, 512)
        if c >= 4:
            eT = eT[::-1]
        maps.append(dict(
            edge_src=np.ascontiguousarray(eT.astype(np.float16)),
            node_embT=np.ascontiguousarray(node_emb[I].T),
            dec_init=np.ascontiguousarray(
                np.stack([edge[I, -1], np.ones(BD, np.float32)]).astype(np.float16)),
        ))
    return maps


def _weights_key(inputs):
    h = hashlib.sha1()
    for k in sorted(inputs):
        if k in ("node_data", "edge_data", "emb"):
            continue
        a = np.asarray(inputs[k])
        h.update(k.encode())
        h.update(a.tobytes())
    return h.hexdigest()


def _get_state():
    if "state" in _CACHE:
        return _CACHE["state"]
    nc = _build_program()
    b2j.install_neuronx_cc_hook()

    partition_name = nc.partition_id_tensor.name if nc.partition_id_tensor else None
    in_names, out_names, out_avals = [], [], []
    for alloc in nc.m.functions[0].allocations:
        if not isinstance(alloc, mybir.MemoryLocationSet):
            continue
        name = alloc.memorylocations[0].name
        if alloc.kind == "ExternalInput":
            if name != partition_name:
                in_names.append(name)
        elif alloc.kind == "ExternalOutput":
            out_names.append(name)
            out_avals.append(jax.core.ShapedArray(
                tuple(alloc.tensor_shape), mybir.dt.np(alloc.dtype)))
    in_names_all = list(in_names) + ([partition_name] if partition_name else [])

    def _body(*args):
        operands = list(args)
        if partition_name is not None:
            operands.append(b2j.partition_id_tensor())
        outs = b2j._bass_exec_p.bind(
            *operands, out_avals=tuple(out_avals), in_names=tuple(in_names_all),
            out_names=tuple(out_names), lowering_input_output_aliases=(),
            sim_require_finite=True, sim_require_nnan=True, nc=nc)
        return tuple(outs)

    devices = jax.devices()[:NC]
    mesh = Mesh(np.asarray(devices), ("core",))
    jitted = jax.jit(
        jax.shard_map(_body, mesh=mesh,
                      in_specs=(PartitionSpec("core"),) * len(in_names),
                      out_specs=(PartitionSpec("core"),) * len(out_names),
                      check_vma=False),
        keep_unused=True)

    # shapes/dtypes of the global (concat over cores) inputs, for lowering
    name_to_alloc = {}
    for alloc in nc.m.functions[0].allocations:
        if isinstance(alloc, mybir.MemoryLocationSet) and alloc.kind == "ExternalInput":
            name_to_alloc[alloc.memorylocations[0].name] = alloc
    sds = [
        jax.ShapeDtypeStruct(
            (NC * name_to_alloc[n].tensor_shape[0], *name_to_alloc[n].tensor_shape[1:]),
            mybir.dt.np(name_to_alloc[n].dtype))
        for n in in_names
    ]
    compiled = jitted.lower(*sds).compile()

    state = dict(
        compiled=compiled, in_names=in_names, out_names=out_names,
        sharding=NamedSharding(mesh, PartitionSpec("core")),
        static_key=None, static_dev={},
    )
    _CACHE["state"] = state
    return state


def _concat(maps, name):
    return np.concatenate([np.asarray(m[name]) for m in maps], axis=0)


def run_device(inputs, trace=False):
    st = _get_state()
    key = _weights_key(inputs)
    if st["static_key"] != key:
        smaps = _prep_static(inputs)
        dev = {}
        for n in smaps[0]:
            dev[n] = jax.device_put(_concat(smaps, n), st["sharding"])
        jax.block_until_ready(list(dev.values()))
        st["static_dev"] = dev
        st["static_key"] = key
    dmaps = _prep_dynamic(inputs)
    args = []
    for n in st["in_names"]:
        if n in DYNAMIC:
            args.append(_concat(dmaps, n))
        else:
            args.append(st["static_dev"][n])
    out_arrs = st["compiled"](*args)
    staged = np.asarray(out_arrs[0]).reshape(NC, 25, 8, BD)  # (core, blk, k, b)
    out = np.zeros((2048, T, 1), np.float32)
    for c in range(NC):
        out[256 * c : 256 * c + 256, :, 0] = staged[c].reshape(T, BD).T
    return out, None


def kernel(**inputs) -> np.ndarray:
    out, _ = run_device(inputs)
    return out
